# revision 1
# baseline (speedup 1.0000x reference)
"""ChebNet (K=3, 4 layers, H=200) on 8 TRN2 NeuronCores.

Strategy: data-parallel over graphs (32 graphs / core, contiguous node
ranges since batch is sorted). The sparse propagation L_hat@h is done as
  gather h[src] rows (dma_gather from a replicated copy in HBM)
  -> segmented matmul against an on-chip-built selection matrix M
     (M[e, d] = w[e] * (dst_local[e] == d)) accumulating per dst tile.
Replication of h across cores between propagations via AllGather.
The Chebyshev recurrence is refactored to avoid the *2/-Tx0 fixups:
  out = Tx0@(W0-W2) + Tx1@W1 + (L Tx1)@(2 W2) + b.
All device compute in bf16 with fp32 PSUM accumulation.
"""

import sys
import types

sys.path.insert(0, "/opt/trn_rl_repo")

import ml_dtypes
import numpy as np

# antenv.axon_hooks shim (lets run_bass_kernel_spmd(trace=True) profile)
try:
    import trn_agent_boot.trn_boot as _tb

    if "antenv.axon_hooks" not in sys.modules:
        _hook = _tb._ntff_profile_via_ctypes("/opt/axon/libaxon_pjrt.so")
        _m = types.ModuleType("antenv.axon_hooks")
        _m.get_axon_ntff_profile_hook = lambda: _hook
        _m.set_axon_ntff_profile_hook = lambda h: None
        sys.modules["antenv.axon_hooks"] = _m
except Exception:
    pass

import concourse.bass as bass
import concourse.mybir as mybir
import concourse.tile as tile
from concourse import bacc
from concourse.bass_utils import run_bass_kernel_spmd
from concourse.masks import make_identity

BF16 = ml_dtypes.bfloat16
NCORES = 8
G = 256
GPC_OUT = G // NCORES  # graphs per core = 32
H = 200
FIN = 64
POOL_GPC = 8  # graphs per pooling gather call
P = 128

_cache = {}


def _wrap_idx(vals):
    """[n] int -> [128, n//16] int16 in dma_gather wrapped+replicated layout."""
    n = len(vals)
    assert n % 16 == 0
    w16 = np.asarray(vals, np.int16).reshape(n // 16, 16).T  # [16, n/16]
    return np.tile(w16, (8, 1))  # [128, n/16]


def _preprocess(x, edge_index, batch, lmax):
    N = x.shape[0]
    E = edge_index.shape[1]
    src = edge_index[0].astype(np.int64)
    dst = edge_index[1].astype(np.int64)
    batch = batch.astype(np.int64)

    # --- edge weights (mirror reference, fp32) ---
    deg = np.bincount(src, minlength=N).astype(np.float32)
    dis = np.where(deg > 0, np.maximum(deg, 1.0) ** -0.5, 0.0).astype(np.float32)
    scale = (2.0 / lmax).astype(np.float32)  # [G]
    w_edge = (-dis[src] * dis[dst] * scale[batch[src]]).astype(np.float32)
    diag = (scale[batch] - 1.0).astype(np.float32)  # [N]

    # --- node partition: core c owns graphs [32c, 32c+32) ---
    node_core = (batch // GPC_OUT).astype(np.int64)
    counts = np.bincount(node_core, minlength=NCORES)
    assert counts.min() > 0
    NL = int(np.ceil((counts.max() + 1) / P) * P)
    NT = NL // P
    core_start = np.zeros(NCORES + 1, np.int64)
    core_start[1:] = np.cumsum(counts)
    slot = np.arange(N) - core_start[node_core]  # local slot (natural order)
    g_row = node_core * NL + slot  # row in gathered tensors

    # --- full edge list incl self edges (i, i, diag_i) ---
    asrc = np.concatenate([src, np.arange(N)])
    adst = np.concatenate([dst, np.arange(N)])
    aw = np.concatenate([w_edge, diag]).astype(np.float32)

    e_core = node_core[adst]  # owning core (by dst)
    e_tile = (slot[adst] >> 7).astype(np.int64)
    e_dl = (slot[adst] & 127).astype(np.int64)
    e_half = (node_core[asrc] >= 4).astype(np.int64)  # 0: rows<4NL, 1: rest
    e_grow = g_row[asrc]

    # group by (core, tile, half); order within group arbitrary
    order = np.lexsort((e_half, e_tile, e_core))
    gkey = ((e_core * NT + e_tile) * 2 + e_half)[order]
    # within-group rank
    grp_start_mask = np.ones(len(gkey), bool)
    grp_start_mask[1:] = gkey[1:] != gkey[:-1]
    grp_idx = np.flatnonzero(grp_start_mask)
    within = np.arange(len(gkey)) - np.repeat(grp_idx, np.diff(np.append(grp_idx, len(gkey))))
    cnts = np.zeros(NCORES * NT * 2, np.int64)
    uk, uc = np.unique(gkey, return_counts=True)
    cnts[uk] = uc
    cntA = cnts.reshape(-1, 2)[:, 0]
    cntB = cnts.reshape(-1, 2)[:, 1]
    cA = int(np.ceil(cntA.max() / P))
    cB = int(np.ceil(cntB.max() / P))
    C = cA + cB

    # slot position of each edge inside the per-core [NT, C*128] array
    pos_in_tile = np.where(gkey % 2 == 0, within, cA * P + within)
    pos = (gkey // 2 % NT) * (C * P) + pos_in_tile
    ecore_sorted = e_core[order]

    idx_arr = np.zeros((NCORES, NT * C * P), np.int64)  # gathered-row index
    dl_arr = np.zeros((NCORES, NT * C * P), np.float32)
    w_arr = np.zeros((NCORES, NT * C * P), np.float32)
    grow_adj = np.where(e_half == 1, e_grow - 4 * NL, e_grow)
    idx_arr[ecore_sorted, pos] = grow_adj[order]
    dl_arr[ecore_sorted, pos] = e_dl[order]
    w_arr[ecore_sorted, pos] = aw[order]
    assert idx_arr.max() < 32768

    # device layouts
    idx_dev, mm_dev = [], []
    for c in range(NCORES):
        a3 = idx_arr[c].reshape(NT, C, P)
        cols = []
        for t in range(NT):
            cols.append(_wrap_idx(a3[t, :cA].reshape(-1)))
            cols.append(_wrap_idx(a3[t, cA:].reshape(-1)))
        idx_dev.append(np.hstack(cols))  # [128, NT*C*8]
        # expanded selection matrices: mm[t, e, k*128+d] = w * (dstloc==d)
        mm = np.zeros((NT, P, C * P), BF16)
        kk = np.arange(NT * C * P)
        t_of = kk // (C * P)
        k_of = (kk % (C * P)) // P
        e_of = kk % P
        mm[t_of, e_of, k_of * P + dl_arr[c].astype(np.int64)] = w_arr[c].astype(BF16)
        mm_dev.append(mm)

    # --- x layouts ---
    xg = np.zeros((NCORES * NL, P), BF16)
    xg[g_row, :FIN] = x.astype(BF16)
    xloc = [np.ascontiguousarray(xg[c * NL : (c + 1) * NL]) for c in range(NCORES)]

    # --- pooling windows ---
    gcnt = np.bincount(batch, minlength=G).astype(np.int64)
    assert gcnt.min() > 0
    GW = int(np.ceil(gcnt.max() / 16) * 16)
    n_calls = GPC_OUT // POOL_GPC  # 4
    pidx = []
    cntr = []
    for c in range(NCORES):
        zrow = int(counts[c])  # first pad slot, rows are zero
        mean_cols, max_cols = [], []
        for call in range(n_calls):
            mvals = np.zeros(POOL_GPC * GW, np.int64)
            xvals = np.zeros(POOL_GPC * GW, np.int64)
            for gg in range(POOL_GPC):
                g_id = c * GPC_OUT + call * POOL_GPC + gg
                # nodes of graph g_id are a contiguous range (batch sorted)
                lo = np.searchsorted(batch, g_id, "left")
                hi = np.searchsorted(batch, g_id, "right")
                rows = slot[lo:hi]
                k = hi - lo
                mvals[gg * GW : gg * GW + k] = rows
                mvals[gg * GW + k : (gg + 1) * GW] = zrow
                xvals[gg * GW : gg * GW + k] = rows
                xvals[gg * GW + k : (gg + 1) * GW] = rows[0]
            mean_cols.append(_wrap_idx(mvals))
            max_cols.append(_wrap_idx(xvals))
        pidx.append(np.hstack(mean_cols + max_cols))  # [128, 8*GPC*GW/16]
        # replicated across all 128 partitions (free-dim per-graph scale)
        cr = (1.0 / np.maximum(gcnt[c * GPC_OUT : (c + 1) * GPC_OUT], 1.0)).astype(
            np.float32
        )
        cntr.append(np.tile(cr.reshape(1, GPC_OUT), (P, 1)))

    return dict(
        NL=NL, NT=NT, C=C, cA=cA, cB=cB, GW=GW,
        idx=idx_dev, mm=mm_dev, xg=xg, xloc=xloc,
        pidx=pidx, cntr=cntr,
    )


def _pack_weights(W1, W2, W3, W4, b1, b2, b3, b4, fc_w, fc_b):
    def cheb_pack(W, kin_chunks):
        # W [3, Fin, 200] -> W' terms [(W0-W2), W1, 2*W2]; pad to [3, kc, 128, 256]
        Wp = np.stack([W[0] - W[2], W[1], 2.0 * W[2]]).astype(np.float32)
        out = np.zeros((3, kin_chunks, P, 256), np.float32)
        fin = W.shape[1]
        for ki in range(kin_chunks):
            lo = ki * P
            hi = min(fin, lo + P)
            if hi > lo:
                out[:, ki, : hi - lo, :H] = Wp[:, lo:hi, :]
        return out.astype(BF16)

    w1 = cheb_pack(W1, 1)
    w2 = cheb_pack(W2, 2)
    w3 = cheb_pack(W3, 2)
    w4 = cheb_pack(W4, 2)
    bvec = np.zeros((P, 4, 2), np.float32)
    for li, b in enumerate([b1, b2, b3, b4]):
        for fo in range(2):
            seg = b[fo * P : min(H, (fo + 1) * P)]
            bvec[: len(seg), li, fo] = seg
    fcw = np.zeros((P, 4, 2), np.float32)
    fcw[:, 0] = fc_w[0:P]
    fcw[: H - P, 1] = fc_w[P:H]
    fcw[:, 2] = fc_w[H : H + P]
    fcw[: H - P, 3] = fc_w[H + P : 2 * H]
    fcb = np.tile(fc_b.astype(np.float32).reshape(1, 2), (GPC_OUT, 1))
    return dict(
        w1=w1, w2=w2, w3=w3, w4=w4, bvec=bvec, fcw=fcw.astype(BF16), fcb=fcb
    )


def _build(NL, NT, C, cA, cB, GW, stage=None):
    """Build the SPMD kernel graph (identical for all cores).

    stage (debug only): stop after stage k and dump an intermediate
    feature-major tile buffer to the extra 'dbg' output.
    """
    F32, BF, I16 = mybir.dt.float32, mybir.dt.bfloat16, mybir.dt.int16
    nc = bacc.Bacc(None, num_devices=NCORES, num_swdge_queues=4)
    rg = [list(range(NCORES))]
    n_calls = GPC_OUT // POOL_GPC

    # inputs
    d_xg = nc.declare_dram_parameter("xg", [NCORES * NL, P], BF, isOutput=False)
    d_xloc = nc.declare_dram_parameter("xloc", [NL, P], BF, isOutput=False)
    d_idx = nc.declare_dram_parameter("idx", [P, NT * C * 8], I16, isOutput=False)
    d_mm = nc.declare_dram_parameter("mm", [NT, P, C * P], BF, isOutput=False)
    d_pidx = nc.declare_dram_parameter(
        "pidx", [P, 2 * n_calls * POOL_GPC * GW // 16], I16, isOutput=False
    )
    d_cntr = nc.declare_dram_parameter("cntr", [P, GPC_OUT], F32, isOutput=False)
    d_w1 = nc.declare_dram_parameter("w1", [3, 1, P, 256], BF, isOutput=False)
    d_w2 = nc.declare_dram_parameter("w2", [3, 2, P, 256], BF, isOutput=False)
    d_w3 = nc.declare_dram_parameter("w3", [3, 2, P, 256], BF, isOutput=False)
    d_w4 = nc.declare_dram_parameter("w4", [3, 2, P, 256], BF, isOutput=False)
    d_bvec = nc.declare_dram_parameter("bvec", [P, 4, 2], F32, isOutput=False)
    d_fcw = nc.declare_dram_parameter("fcw", [P, 4, 2], BF, isOutput=False)
    d_fcb = nc.declare_dram_parameter("fcb", [GPC_OUT, 2], F32, isOutput=False)
    d_out = nc.declare_dram_parameter("out", [GPC_OUT, 2], F32, isOutput=True)
    d_dbg = (
        nc.declare_dram_parameter("dbg", [P, 2 * NT * P], BF, isOutput=True)
        if stage is not None
        else None
    )

    # internal DRAM
    bounce = nc.dram_tensor("bounce", [NL, 256], BF)
    bounce_s = nc.dram_tensor("bounce_s", [NL, P], BF)
    hg = nc.dram_tensor("hg", [NCORES * NL, 256], BF, addr_space="Shared")
    t1g = nc.dram_tensor("t1g", [NCORES * NL, 256], BF, addr_space="Shared")
    t1g_s = nc.dram_tensor("t1g_s", [NCORES * NL, P], BF, addr_space="Shared")

    with tile.TileContext(nc) as tc:
        with (
            tc.tile_pool(name="const", bufs=1) as cp,
            tc.tile_pool(name="big", bufs=1) as bigp,
            tc.tile_pool(name="work", bufs=3) as wp,
            tc.tile_pool(name="ypool", bufs=2) as yp,
            tc.tile_pool(name="ygath", bufs=3) as ygp,
            tc.tile_pool(name="mpool", bufs=3) as mp,
            tc.tile_pool(name="psA", bufs=3, space="PSUM") as psA,
            tc.tile_pool(name="psB", bufs=2, space="PSUM") as psB,
            tc.tile_pool(name="psC", bufs=2, space="PSUM") as psC,
            tc.tile_pool(name="psD", bufs=1, space="PSUM") as psD,
        ):
            # ---- resident constants ----
            idx_sb = cp.tile([P, NT * C * 8], I16)
            nc.sync.dma_start(out=idx_sb[:], in_=d_idx.ap())
            pidx_sb = cp.tile([P, 2 * n_calls * POOL_GPC * GW // 16], I16)
            nc.sync.dma_start(out=pidx_sb[:], in_=d_pidx.ap())
            w_sb = {}
            for nm, dp, kc in (("w1", d_w1, 1), ("w2", d_w2, 2), ("w3", d_w3, 2), ("w4", d_w4, 2)):
                t = cp.tile([P, 3, kc, 256], BF, tag=nm)
                nc.sync.dma_start(
                    out=t[:], in_=dp.ap().rearrange("t k p f -> p t k f")
                )
                w_sb[nm] = t
            bvec_sb = cp.tile([P, 4, 2], F32)
            nc.sync.dma_start(out=bvec_sb[:], in_=d_bvec.ap())
            fcw_sb = cp.tile([P, 4, 2], BF)
            nc.sync.dma_start(out=fcw_sb[:], in_=d_fcw.ap())
            fcb_sb = cp.tile([GPC_OUT, 2], F32)
            nc.sync.dma_start(out=fcb_sb[:], in_=d_fcb.ap())
            cntr_sb = cp.tile([P, GPC_OUT], F32)
            nc.sync.dma_start(out=cntr_sb[:], in_=d_cntr.ap())

            ident = cp.tile([P, P], BF)
            make_identity(nc, ident[:])

            # ---- feature-major locals ----
            hT = [bigp.tile([P, 2, NT * P], BF, tag=f"hT{i}", name=f"hT{i}") for i in range(2)]
            t1T = bigp.tile([P, 2, NT * P], BF, tag="t1T")
            p2T = bigp.tile([P, 2, NT * P], BF, tag="p2T")
            for buf in (hT[0], hT[1], t1T, p2T):
                nc.vector.memset(buf[:], 0.0)
            zt = cp.tile([P, 256], BF)
            nc.vector.memset(zt[:], 0.0)
            for t in range(NT):
                nc.sync.dma_start(
                    out=bounce.ap()[t * P : (t + 1) * P, :], in_=zt[:]
                )

            # x -> xT (= hT[0] chunk 0)
            for t in range(NT):
                xt = wp.tile([P, P], BF, tag="xload")
                nc.sync.dma_start(out=xt[:], in_=d_xloc.ap()[t * P : (t + 1) * P, :])
                pt = psB.tile([P, P], BF, tag="tp")
                nc.tensor.transpose(pt[:], xt[:], ident[:])
                nc.vector.tensor_copy(out=hT[0][:, 0, t * P : (t + 1) * P], in_=pt[:])

            def prop(src_dram, EW, NW, outT, bounce_dram):
                """outT[f, n] = sum_e w[e] h_src[e, f] per dst tile; optionally
                also write node-major rows to bounce_dram."""
                lo = src_dram.ap()[0 : 4 * NL, :]
                hi = src_dram.ap()[4 * NL : 8 * NL, :]
                for t in range(NT):
                    y = ygp.tile([P, C, EW], BF, tag="Y")
                    nc.gpsimd.dma_gather(
                        out_ap=y[:, 0:cA, :],
                        in_ap=lo,
                        idxs_ap=idx_sb[:, t * C * 8 : t * C * 8 + cA * 8],
                        num_idxs=cA * P,
                        num_idxs_reg=cA * P,
                        elem_size=EW,
                        single_packet=False,
                        queue_num=t % 4,
                    )
                    nc.gpsimd.dma_gather(
                        out_ap=y[:, cA:C, :],
                        in_ap=hi,
                        idxs_ap=idx_sb[:, t * C * 8 + cA * 8 : (t + 1) * C * 8],
                        num_idxs=cB * P,
                        num_idxs_reg=cB * P,
                        elem_size=EW,
                        single_packet=False,
                        queue_num=(t + 2) % 4,
                    )
                    mt = mp.tile([P, C, P], BF, tag="mt")
                    nc.sync.dma_start(
                        out=mt[:],
                        in_=d_mm.ap()[t].rearrange("e (k d) -> e k d", d=P),
                    )
                    acc = psA.tile([P, NW], F32, tag="acc")
                    for k in range(C):
                        nc.tensor.matmul(
                            acc[:],
                            lhsT=mt[:, k, :],
                            rhs=y[:, k, 0:NW],
                            start=(k == 0),
                            stop=(k == C - 1),
                        )
                    nm = wp.tile([P, NW], BF, tag="nm")
                    nc.vector.tensor_copy(out=nm[:], in_=acc[:])
                    nfc = (NW + P - 1) // P
                    for fc in range(nfc):
                        w_fc = min(P, NW - fc * P)
                        pt = psB.tile([P, P], BF, tag="tp")
                        nc.tensor.transpose(
                            pt[:w_fc, :],
                            nm[:, fc * P : fc * P + w_fc],
                            ident[:],
                        )
                        nc.vector.tensor_copy(
                            out=outT[:w_fc, fc, t * P : (t + 1) * P], in_=pt[:w_fc, :]
                        )
                    if bounce_dram is not None:
                        nc.sync.dma_start(
                            out=bounce_dram.ap()[t * P : (t + 1) * P, 0:NW],
                            in_=nm[:],
                        )

            def dense(l_idx, wt, kc, inT0, h_out):
                """h_out = relu(Tx0@W'0 + Tx1@W'1 + P2@W'2 + b), feature-major;
                also write node-major tiles to bounce."""
                terms = [(inT0, 0), (t1T, 1), (p2T, 2)]
                for t in range(NT):
                    nm = wp.tile([P, 256], BF, tag="nm")
                    for fo in range(2):
                        pd = psC.tile([P, P], F32, tag="pd")
                        n_mm = len(terms) * kc
                        i_mm = 0
                        for inT, term in terms:
                            for ki in range(kc):
                                nc.tensor.matmul(
                                    pd[:],
                                    lhsT=w_sb[wt][:, term, ki, fo * P : (fo + 1) * P],
                                    rhs=inT[:, ki, t * P : (t + 1) * P],
                                    start=(i_mm == 0),
                                    stop=(i_mm == n_mm - 1),
                                )
                                i_mm += 1
                        nc.scalar.activation(
                            h_out[:, fo, t * P : (t + 1) * P],
                            pd[:],
                            mybir.ActivationFunctionType.Relu,
                            bias=bvec_sb[:, l_idx, fo : fo + 1],
                        )
                        pt = psB.tile([P, P], BF, tag="tp")
                        nc.tensor.transpose(
                            pt[:], h_out[:, fo, t * P : (t + 1) * P], ident[:]
                        )
                        nc.vector.tensor_copy(
                            out=nm[:, fo * P : (fo + 1) * P], in_=pt[:]
                        )
                    nc.sync.dma_start(
                        out=bounce.ap()[t * P : (t + 1) * P, :], in_=nm[:]
                    )

            def allgather(src, dst):
                nc.gpsimd.collective_compute(
                    "AllGather",
                    mybir.AluOpType.bypass,
                    replica_groups=rg,
                    ins=[src.ap().opt()],
                    outs=[dst.ap().opt()],
                )

            # ================= layer 1 (input x, width 64->200) =================
            if stage is not None:
                # debug: run a prefix of the pipeline, dump one tT buffer
                dbg_src = hT[0]
                if stage >= 2:
                    prop(d_xg, P, FIN, t1T, bounce_s)
                    dbg_src = t1T
                if stage >= 3:
                    allgather(bounce_s, t1g_s)
                    prop(t1g_s, P, FIN, p2T, None)
                    dbg_src = p2T
                if stage >= 4:
                    dense(0, "w1", 1, hT[0], hT[1])
                    dbg_src = hT[1]
                if stage >= 5:
                    allgather(bounce, hg)
                    prop(hg, 256, H, t1T, bounce)
                    dbg_src = t1T
                if stage >= 6:
                    allgather(bounce, t1g)
                    prop(t1g, 256, H, p2T, None)
                    dbg_src = p2T
                nc.sync.dma_start(
                    out=d_dbg.ap().rearrange("p (c n) -> p c n", c=2), in_=dbg_src[:]
                )
            else:
                prop(d_xg, P, FIN, t1T, bounce_s)
                allgather(bounce_s, t1g_s)
                prop(t1g_s, P, FIN, p2T, None)
                dense(0, "w1", 1, hT[0], hT[1])
                allgather(bounce, hg)

                # ================= layers 2..4 =================
                cur = 1
                for li, wt in ((1, "w2"), (2, "w3"), (3, "w4")):
                    prop(hg, 256, H, t1T, bounce)
                    allgather(bounce, t1g)
                    prop(t1g, 256, H, p2T, None)
                    dense(li, wt, 2, hT[cur], hT[1 - cur])
                    cur = 1 - cur
                    if li < 3:
                        allgather(bounce, hg)

            if stage is None:
                # ================= pooling + fc + log_softmax =================
                red = {}
                for typ in range(2):  # 0: mean(sum), 1: max
                    rT = bigp.tile([P, 2, GPC_OUT], F32, tag=f"red{typ}")
                    red[typ] = rT
                    for call in range(n_calls):
                        py = yp.tile([P, 2, POOL_GPC * GW], BF, tag="poolY")
                        base = (typ * n_calls + call) * (POOL_GPC * GW // 16)
                        nc.gpsimd.dma_gather(
                            out_ap=py[:],
                            in_ap=bounce.ap(),
                            idxs_ap=pidx_sb[:, base : base + POOL_GPC * GW // 16],
                            num_idxs=POOL_GPC * GW,
                            num_idxs_reg=POOL_GPC * GW,
                            elem_size=256,
                            transpose=True,
                            single_packet=False,
                        )
                        for gg in range(POOL_GPC):
                            gcol = call * POOL_GPC + gg
                            for fc in range(2):
                                nc.vector.tensor_reduce(
                                    out=rT[:, fc, gcol : gcol + 1],
                                    in_=py[:, fc, gg * GW : (gg + 1) * GW],
                                    axis=mybir.AxisListType.X,
                                    op=mybir.AluOpType.add if typ == 0 else mybir.AluOpType.max,
                                )
                # mean scale + cast to bf16 lhsT tiles
                pool_bf = bigp.tile([P, 4, GPC_OUT], BF, tag="poolbf")
                for fc in range(2):
                    nc.vector.tensor_tensor(
                        out=pool_bf[:, fc, :],
                        in0=red[0][:, fc, :],
                        in1=cntr_sb[:, :],
                        op=mybir.AluOpType.mult,
                    )
                    nc.vector.tensor_copy(out=pool_bf[:, 2 + fc, :], in_=red[1][:, fc, :])
                pfc = psD.tile([GPC_OUT, 2], F32, tag="pfc")
                for j in range(4):
                    nc.tensor.matmul(
                        pfc[:],
                        lhsT=pool_bf[:, j, :],
                        rhs=fcw_sb[:, j, :],
                        start=(j == 0),
                        stop=(j == 3),
                    )
                z = wp.tile([GPC_OUT, 2], F32, tag="z")
                nc.vector.tensor_tensor(
                    out=z[:], in0=pfc[:], in1=fcb_sb[:, :],
                    op=mybir.AluOpType.add,
                )
                mx = wp.tile([GPC_OUT, 1], F32, tag="mx")
                nc.vector.tensor_reduce(
                    out=mx[:], in_=z[:], axis=mybir.AxisListType.X, op=mybir.AluOpType.max
                )
                zm = wp.tile([GPC_OUT, 2], F32, tag="zm")
                nc.vector.tensor_scalar(
                    zm[:], z[:], mx[:], None, mybir.AluOpType.subtract
                )
                ez = wp.tile([GPC_OUT, 2], F32, tag="ez")
                nc.scalar.activation(ez[:], zm[:], mybir.ActivationFunctionType.Exp)
                sz = wp.tile([GPC_OUT, 1], F32, tag="sz")
                nc.vector.tensor_reduce(
                    out=sz[:], in_=ez[:], axis=mybir.AxisListType.X, op=mybir.AluOpType.add
                )
                lz = wp.tile([GPC_OUT, 1], F32, tag="lz")
                nc.scalar.activation(lz[:], sz[:], mybir.ActivationFunctionType.Ln)
                oz = wp.tile([GPC_OUT, 2], F32, tag="oz")
                nc.vector.tensor_scalar(
                    oz[:], zm[:], lz[:], None, mybir.AluOpType.subtract
                )
                nc.sync.dma_start(out=d_out.ap(), in_=oz[:])

    nc.finalize()
    return nc


def kernel(**inputs):
    x = np.asarray(inputs["x"], np.float32)
    edge_index = np.asarray(inputs["edge_index"])
    batch = np.asarray(inputs["batch"])
    lmax = np.asarray(inputs["lmax"], np.float32)

    pp = _preprocess(x, edge_index, batch, lmax)
    wts = _pack_weights(
        np.asarray(inputs["W1"], np.float32), np.asarray(inputs["W2"], np.float32),
        np.asarray(inputs["W3"], np.float32), np.asarray(inputs["W4"], np.float32),
        np.asarray(inputs["b1"], np.float32), np.asarray(inputs["b2"], np.float32),
        np.asarray(inputs["b3"], np.float32), np.asarray(inputs["b4"], np.float32),
        np.asarray(inputs["fc_w"], np.float32), np.asarray(inputs["fc_b"], np.float32),
    )

    key = (pp["NL"], pp["NT"], pp["C"], pp["cA"], pp["cB"], pp["GW"])
    if key not in _cache:
        _cache[key] = _build(*key)
    nc = _cache[key]

    shared = dict(
        xg=pp["xg"],
        w1=wts["w1"], w2=wts["w2"], w3=wts["w3"], w4=wts["w4"],
        bvec=wts["bvec"], fcw=wts["fcw"], fcb=wts["fcb"],
    )
    in_maps = [
        dict(
            shared,
            xloc=pp["xloc"][c], idx=pp["idx"][c], mm=pp["mm"][c],
            pidx=pp["pidx"][c], cntr=pp["cntr"][c],
        )
        for c in range(NCORES)
    ]
    trace = bool(int(__import__("os").environ.get("KERNEL_TRACE", "0")))
    res = run_bass_kernel_spmd(nc, in_maps, list(range(NCORES)), trace=trace)
    if trace:
        kernel.last_exec_time_ns = res.exec_time_ns
        kernel.last_results = res
    out = np.concatenate([res.results[c]["out"] for c in range(NCORES)], axis=0)
    return out.astype(np.float32)


kernel.last_exec_time_ns = None



# revision 2
# speedup vs baseline: 1.0564x; 1.0564x over previous
"""ChebNet (K=3, 4 layers, H=200) on 8 TRN2 NeuronCores.

v2: fp8(e4m3) gather tables + fp8 selection matrices (halves prop HBM
traffic vs bf16), slot-half table split (int16-indexable AND allows each
AllGather to be issued as two halves pipelined under the producing loop),
2-tile gather batching (halves SWDGE call overhead), fp8 AllGathers.

Sparse prop L_hat@h per dst tile: dma_gather h[src] rows from a
replicated HBM table -> matmul against streamed selection matrices
M[e, d] = w[e] * (dst_local[e] == d), fp32 PSUM accumulation.
Chebyshev refactor: out = Tx0@(W0-W2) + Tx1@W1 + (L Tx1)@(2 W2) + b.
Dense layers in bf16; pooling reads an exact bf16 copy of the last h.
"""

import sys
import types

sys.path.insert(0, "/opt/trn_rl_repo")

import ml_dtypes
import numpy as np

# antenv.axon_hooks shim (lets run_bass_kernel_spmd(trace=True) profile)
try:
    import trn_agent_boot.trn_boot as _tb

    if "antenv.axon_hooks" not in sys.modules:
        _hook = _tb._ntff_profile_via_ctypes("/opt/axon/libaxon_pjrt.so")
        _m = types.ModuleType("antenv.axon_hooks")
        _m.get_axon_ntff_profile_hook = lambda: _hook
        _m.set_axon_ntff_profile_hook = lambda h: None
        sys.modules["antenv.axon_hooks"] = _m
except Exception:
    pass

import concourse.bass as bass
import concourse.mybir as mybir
import concourse.tile as tile
from concourse import bacc
from concourse.bass_utils import run_bass_kernel_spmd
from concourse.masks import make_identity

BF16 = ml_dtypes.bfloat16
FP8 = ml_dtypes.float8_e4m3  # TRN float8e4
NCORES = 8
G = 256
GPC_OUT = G // NCORES  # graphs per core = 32
H = 200
FIN = 64
POOL_GPC = 8  # graphs per pooling gather call
P = 128

_cache = {}


def _wrap_idx(vals):
    """[n] int -> [128, n//16] int16 in dma_gather wrapped+replicated layout."""
    n = len(vals)
    assert n % 16 == 0
    w16 = np.asarray(vals, np.int16).reshape(n // 16, 16).T  # [16, n/16]
    return np.tile(w16, (8, 1))  # [128, n/16]


def _preprocess(x, edge_index, batch, lmax):
    N = x.shape[0]
    src = edge_index[0].astype(np.int64)
    dst = edge_index[1].astype(np.int64)
    batch = batch.astype(np.int64)

    # --- edge weights (mirror reference, fp32) ---
    deg = np.bincount(src, minlength=N).astype(np.float32)
    dis = np.where(deg > 0, np.maximum(deg, 1.0) ** -0.5, 0.0).astype(np.float32)
    scale = (2.0 / lmax).astype(np.float32)  # [G]
    w_edge = (-dis[src] * dis[dst] * scale[batch[src]]).astype(np.float32)
    diag = (scale[batch] - 1.0).astype(np.float32)  # [N]

    # --- node partition: core c owns graphs [32c, 32c+32) ---
    node_core = (batch // GPC_OUT).astype(np.int64)
    counts = np.bincount(node_core, minlength=NCORES)
    assert counts.min() > 0
    NL = int(np.ceil((counts.max() + 1) / P) * P)
    NT = NL // P
    assert NT % 2 == 0
    NSTEP = NT // 2
    # slot-half split: table A = slots [0, HLA), table B = [HLA, NL)
    TA = (NT // 2 + 1) if (NT // 2) % 2 else NT // 2  # tiles in half A (even step boundary)
    TA = 26 if NT == 50 else TA
    HLA = TA * P
    HLB = NL - HLA
    assert 8 * HLA < 32768 and 8 * HLB < 32768
    core_start = np.zeros(NCORES + 1, np.int64)
    core_start[1:] = np.cumsum(counts)
    slot = np.arange(N) - core_start[node_core]  # local slot (natural order)
    half = (slot >= HLA).astype(np.int64)  # which table
    # row within table: A: c*HLA + s ; B: c*HLB + (s - HLA)
    g_row = np.where(half == 0, node_core * HLA + slot,
                     node_core * HLB + (slot - HLA))

    # --- full edge list incl self edges (i, i, diag_i) ---
    asrc = np.concatenate([src, np.arange(N)])
    adst = np.concatenate([dst, np.arange(N)])
    aw = np.concatenate([w_edge, diag]).astype(np.float32)

    e_core = node_core[adst]  # owning core (by dst)
    e_tile = (slot[adst] >> 7).astype(np.int64)
    e_dl = (slot[adst] & 127).astype(np.int64)
    e_half = half[asrc]  # src table
    e_grow = g_row[asrc]

    # group by (core, tile, half); order within group arbitrary
    order = np.lexsort((e_half, e_tile, e_core))
    gkey = ((e_core * NT + e_tile) * 2 + e_half)[order]
    grp_start_mask = np.ones(len(gkey), bool)
    grp_start_mask[1:] = gkey[1:] != gkey[:-1]
    grp_idx = np.flatnonzero(grp_start_mask)
    within = np.arange(len(gkey)) - np.repeat(
        grp_idx, np.diff(np.append(grp_idx, len(gkey)))
    )
    cnts = np.zeros(NCORES * NT * 2, np.int64)
    uk, uc = np.unique(gkey, return_counts=True)
    cnts[uk] = uc
    cntA = cnts.reshape(-1, 2)[:, 0]
    cntB = cnts.reshape(-1, 2)[:, 1]
    cA = int(np.ceil(cntA.max() / P))
    cB = int(np.ceil(cntB.max() / P))
    C = cA + cB

    # slot position: step q covers tiles (2q, 2q+1); chunk layout per step:
    #   [0, 2cA): A-half (tile 2q chunks 0..cA-1, tile 2q+1 chunks cA..2cA-1)
    #   [2cA, 2C): B-half (tile 2q then 2q+1)
    t_of_e = gkey // 2 % NT
    sub_of_e = t_of_e % 2
    q_of_e = t_of_e // 2
    h_of_e = gkey % 2
    chunk = np.where(
        h_of_e == 0,
        sub_of_e * cA + within // P,
        2 * cA + sub_of_e * cB + within // P,
    )
    pos = (q_of_e * 2 * C + chunk) * P + within % P
    ecore_sorted = e_core[order]

    SLOTS = NSTEP * 2 * C * P  # == NT * C * P
    idx_arr = np.zeros((NCORES, SLOTS), np.int64)  # gathered-row index
    dl_arr = np.zeros((NCORES, SLOTS), np.int64)
    w_arr = np.zeros((NCORES, SLOTS), np.float32)
    idx_arr[ecore_sorted, pos] = e_grow[order]
    dl_arr[ecore_sorted, pos] = e_dl[order]
    w_arr[ecore_sorted, pos] = aw[order]
    assert idx_arr.max() < 32768

    # device layouts
    idx_dev, mm_dev = [], []
    for c in range(NCORES):
        a3 = idx_arr[c].reshape(NSTEP, 2 * C, P)
        cols = []
        for q in range(NSTEP):
            cols.append(_wrap_idx(a3[q, : 2 * cA].reshape(-1)))
            cols.append(_wrap_idx(a3[q, 2 * cA :].reshape(-1)))
        idx_dev.append(np.hstack(cols))  # [128, NSTEP*2C*8]
        # selection matrices: mm[q, e, ch*128+d] = w * (dstloc==d)
        mm = np.zeros((NSTEP, P, 2 * C * P), FP8)
        kk = np.arange(SLOTS)
        q_of = kk // (2 * C * P)
        ch_of = (kk % (2 * C * P)) // P
        e_of = kk % P
        mm[q_of, e_of, ch_of * P + dl_arr[c]] = w_arr[c].astype(FP8)
        mm_dev.append(mm)

    # --- x tables (fp8, 256-wide rows, 64 used) + local node-major x (bf16) ---
    xgA = np.zeros((NCORES * HLA, 256), FP8)
    xgB = np.zeros((NCORES * HLB, 256), FP8)
    x8 = x.astype(FP8)
    selA = half == 0
    xgA[g_row[selA], :FIN] = x8[selA]
    xgB[g_row[~selA], :FIN] = x8[~selA]
    xg_nm = np.zeros((NCORES * NL, P), BF16)
    xg_nm[node_core * NL + slot, :FIN] = x.astype(BF16)
    xloc = [np.ascontiguousarray(xg_nm[c * NL : (c + 1) * NL]) for c in range(NCORES)]

    # --- pooling windows (read bf16 bounce_pool, local rows) ---
    gcnt = np.bincount(batch, minlength=G).astype(np.int64)
    assert gcnt.min() > 0
    GW = int(np.ceil(gcnt.max() / 16) * 16)
    n_calls = GPC_OUT // POOL_GPC  # 4
    pidx = []
    cntr = []
    for c in range(NCORES):
        zrow = int(counts[c])  # first pad slot, rows are zero
        mean_cols, max_cols = [], []
        for call in range(n_calls):
            mvals = np.zeros(POOL_GPC * GW, np.int64)
            xvals = np.zeros(POOL_GPC * GW, np.int64)
            for gg in range(POOL_GPC):
                g_id = c * GPC_OUT + call * POOL_GPC + gg
                lo = np.searchsorted(batch, g_id, "left")
                hi = np.searchsorted(batch, g_id, "right")
                rows = slot[lo:hi]
                k = hi - lo
                mvals[gg * GW : gg * GW + k] = rows
                mvals[gg * GW + k : (gg + 1) * GW] = zrow
                xvals[gg * GW : gg * GW + k] = rows
                xvals[gg * GW + k : (gg + 1) * GW] = rows[0]
            mean_cols.append(_wrap_idx(mvals))
            max_cols.append(_wrap_idx(xvals))
        pidx.append(np.hstack(mean_cols + max_cols))  # [128, 8*PGC*GW/16]
        cr = (1.0 / np.maximum(gcnt[c * GPC_OUT : (c + 1) * GPC_OUT], 1.0)).astype(
            np.float32
        )
        cntr.append(np.tile(cr.reshape(1, GPC_OUT), (P, 1)))

    return dict(
        NL=NL, NT=NT, NSTEP=NSTEP, C=C, cA=cA, cB=cB, GW=GW, TA=TA,
        HLA=HLA, HLB=HLB,
        idx=idx_dev, mm=mm_dev, xgA=xgA, xgB=xgB, xloc=xloc,
        pidx=pidx, cntr=cntr,
    )


def _pack_weights(W1, W2, W3, W4, b1, b2, b3, b4, fc_w, fc_b):
    def cheb_pack(W, kin_chunks):
        # W [3, Fin, 200] -> W' terms [(W0-W2), W1, 2*W2]; pad to [3, kc, 128, 256]
        Wp = np.stack([W[0] - W[2], W[1], 2.0 * W[2]]).astype(np.float32)
        out = np.zeros((3, kin_chunks, P, 256), np.float32)
        fin = W.shape[1]
        for ki in range(kin_chunks):
            lo = ki * P
            hi = min(fin, lo + P)
            if hi > lo:
                out[:, ki, : hi - lo, :H] = Wp[:, lo:hi, :]
        return out.astype(BF16)

    w1 = cheb_pack(W1, 1)
    w2 = cheb_pack(W2, 2)
    w3 = cheb_pack(W3, 2)
    w4 = cheb_pack(W4, 2)
    bvec = np.zeros((P, 4, 2), np.float32)
    for li, b in enumerate([b1, b2, b3, b4]):
        for fo in range(2):
            seg = b[fo * P : min(H, (fo + 1) * P)]
            bvec[: len(seg), li, fo] = seg
    fcw = np.zeros((P, 4, 2), np.float32)
    fcw[:, 0] = fc_w[0:P]
    fcw[: H - P, 1] = fc_w[P:H]
    fcw[:, 2] = fc_w[H : H + P]
    fcw[: H - P, 3] = fc_w[H + P : 2 * H]
    fcb = np.tile(fc_b.astype(np.float32).reshape(1, 2), (GPC_OUT, 1))
    return dict(
        w1=w1, w2=w2, w3=w3, w4=w4, bvec=bvec, fcw=fcw.astype(BF16), fcb=fcb
    )


def _build(NL, NT, NSTEP, C, cA, cB, GW, TA, HLA, HLB):
    """Build the SPMD kernel graph (identical for all cores)."""
    F32, BF, F8, I16 = (
        mybir.dt.float32, mybir.dt.bfloat16, mybir.dt.float8e4, mybir.dt.int16
    )
    nc = bacc.Bacc(None, num_devices=NCORES, num_swdge_queues=4)
    rg = [list(range(NCORES))]
    n_calls = GPC_OUT // POOL_GPC

    # inputs
    d_xgA = nc.declare_dram_parameter("xgA", [NCORES * HLA, 256], F8, isOutput=False)
    d_xgB = nc.declare_dram_parameter("xgB", [NCORES * HLB, 256], F8, isOutput=False)
    d_xloc = nc.declare_dram_parameter("xloc", [NL, P], BF, isOutput=False)
    d_idx = nc.declare_dram_parameter("idx", [P, NSTEP * 2 * C * 8], I16, isOutput=False)
    d_mm = nc.declare_dram_parameter("mm", [NSTEP, P, 2 * C * P], F8, isOutput=False)
    d_pidx = nc.declare_dram_parameter(
        "pidx", [P, 2 * n_calls * POOL_GPC * GW // 16], I16, isOutput=False
    )
    d_cntr = nc.declare_dram_parameter("cntr", [P, GPC_OUT], F32, isOutput=False)
    d_w1 = nc.declare_dram_parameter("w1", [3, 1, P, 256], BF, isOutput=False)
    d_w2 = nc.declare_dram_parameter("w2", [3, 2, P, 256], BF, isOutput=False)
    d_w3 = nc.declare_dram_parameter("w3", [3, 2, P, 256], BF, isOutput=False)
    d_w4 = nc.declare_dram_parameter("w4", [3, 2, P, 256], BF, isOutput=False)
    d_bvec = nc.declare_dram_parameter("bvec", [P, 4, 2], F32, isOutput=False)
    d_fcw = nc.declare_dram_parameter("fcw", [P, 4, 2], BF, isOutput=False)
    d_fcb = nc.declare_dram_parameter("fcb", [GPC_OUT, 2], F32, isOutput=False)
    d_out = nc.declare_dram_parameter("out", [GPC_OUT, 2], F32, isOutput=True)

    # internal DRAM
    bounce = nc.dram_tensor("bounce", [NL, 256], F8)  # h or t1 (fp8 table rows)
    bounce_pool = nc.dram_tensor("bounce_pool", [NL, 256], BF)  # last h, exact
    hgA = nc.dram_tensor("hgA", [NCORES * HLA, 256], F8, addr_space="Shared")
    hgB = nc.dram_tensor("hgB", [NCORES * HLB, 256], F8, addr_space="Shared")
    t1gA = nc.dram_tensor("t1gA", [NCORES * HLA, 256], F8, addr_space="Shared")
    t1gB = nc.dram_tensor("t1gB", [NCORES * HLB, 256], F8, addr_space="Shared")

    with tile.TileContext(nc) as tc:
        with (
            tc.tile_pool(name="const", bufs=1) as cp,
            tc.tile_pool(name="big", bufs=1) as bigp,
            tc.tile_pool(name="work", bufs=3) as wp,
            tc.tile_pool(name="w8", bufs=3) as w8p,
            tc.tile_pool(name="ypool", bufs=2) as yp,
            tc.tile_pool(name="ygath", bufs=3) as ygp,
            tc.tile_pool(name="mpool", bufs=3) as mp,
            tc.tile_pool(name="psA", bufs=3, space="PSUM") as psA,
            tc.tile_pool(name="psB", bufs=2, space="PSUM") as psB,
            tc.tile_pool(name="psC", bufs=2, space="PSUM") as psC,
            tc.tile_pool(name="psD", bufs=1, space="PSUM") as psD,
        ):
            # ---- resident constants ----
            idx_sb = cp.tile([P, NSTEP * 2 * C * 8], I16)
            nc.sync.dma_start(out=idx_sb[:], in_=d_idx.ap())
            pidx_sb = cp.tile([P, 2 * n_calls * POOL_GPC * GW // 16], I16)
            nc.sync.dma_start(out=pidx_sb[:], in_=d_pidx.ap())
            w_sb = {}
            for nm_, dp, kc in (
                ("w1", d_w1, 1), ("w2", d_w2, 2), ("w3", d_w3, 2), ("w4", d_w4, 2)
            ):
                t = cp.tile([P, 3, kc, 256], BF, tag=nm_)
                nc.sync.dma_start(
                    out=t[:], in_=dp.ap().rearrange("t k p f -> p t k f")
                )
                w_sb[nm_] = t
            bvec_sb = cp.tile([P, 4, 2], F32)
            nc.sync.dma_start(out=bvec_sb[:], in_=d_bvec.ap())
            fcw_sb = cp.tile([P, 4, 2], BF)
            nc.sync.dma_start(out=fcw_sb[:], in_=d_fcw.ap())
            fcb_sb = cp.tile([GPC_OUT, 2], F32)
            nc.sync.dma_start(out=fcb_sb[:], in_=d_fcb.ap())
            cntr_sb = cp.tile([P, GPC_OUT], F32)
            nc.sync.dma_start(out=cntr_sb[:], in_=d_cntr.ap())

            ident = cp.tile([P, P], BF)
            make_identity(nc, ident[:])

            # ---- feature-major locals ----
            hT = [
                bigp.tile([P, 2, NT * P], BF, tag=f"hT{i}", name=f"hT{i}")
                for i in range(2)
            ]
            t1T = bigp.tile([P, 2, NT * P], BF, tag="t1T")
            p2T = bigp.tile([P, 2, NT * P], BF, tag="p2T")
            for buf in (hT[0], hT[1], t1T, p2T):
                nc.vector.memset(buf[:], 0.0)
            # zero bounce_pool (mean-pool pad rows must read 0)
            zt = cp.tile([P, 256], BF)
            nc.vector.memset(zt[:], 0.0)
            for t in range(NT):
                nc.sync.dma_start(
                    out=bounce_pool.ap()[t * P : (t + 1) * P, :], in_=zt[:]
                )

            # x -> xT (= hT[0] chunk 0)
            for t in range(NT):
                xt = wp.tile([P, P], BF, tag="xload")
                nc.sync.dma_start(out=xt[:], in_=d_xloc.ap()[t * P : (t + 1) * P, :])
                pt = psB.tile([P, P], BF, tag="tp")
                nc.tensor.transpose(pt[:], xt[:], ident[:])
                nc.vector.tensor_copy(out=hT[0][:, 0, t * P : (t + 1) * P], in_=pt[:])

            def allgather_half(src_dram, dst_dram, half_id):
                lo = 0 if half_id == 0 else HLA
                hi = HLA if half_id == 0 else NL
                nc.gpsimd.collective_compute(
                    "AllGather",
                    mybir.AluOpType.bypass,
                    replica_groups=rg,
                    ins=[bounce.ap()[lo:hi, :].opt()] if src_dram is None
                    else [src_dram.ap()[lo:hi, :].opt()],
                    outs=[dst_dram.ap().opt()],
                )

            def prop(tabA, tabB, NW, outT, write_bounce, ag_hooks=None):
                """outT[f, n] = sum_e w[e] h_src[e, f] per dst tile (+ optional
                fp8 node-major write to bounce). ag_hooks: {step: fn} called
                after that step's tiles complete."""
                for q in range(NSTEP):
                    y = ygp.tile([P, 2 * C, 256], F8, tag="Y")
                    nc.gpsimd.dma_gather(
                        out_ap=y[:, 0 : 2 * cA, :],
                        in_ap=tabA.ap(),
                        idxs_ap=idx_sb[:, q * 2 * C * 8 : q * 2 * C * 8 + 2 * cA * 8],
                        num_idxs=2 * cA * P,
                        num_idxs_reg=2 * cA * P,
                        elem_size=256,
                        single_packet=False,
                        queue_num=q % 4,
                    )
                    nc.gpsimd.dma_gather(
                        out_ap=y[:, 2 * cA : 2 * C, :],
                        in_ap=tabB.ap(),
                        idxs_ap=idx_sb[
                            :, q * 2 * C * 8 + 2 * cA * 8 : (q + 1) * 2 * C * 8
                        ],
                        num_idxs=2 * cB * P,
                        num_idxs_reg=2 * cB * P,
                        elem_size=256,
                        single_packet=False,
                        queue_num=(q + 2) % 4,
                    )
                    mt = mp.tile([P, 2 * C, P], F8, tag="mt")
                    nc.sync.dma_start(
                        out=mt[:],
                        in_=d_mm.ap()[q].rearrange("e (k d) -> e k d", d=P),
                    )
                    for sub in range(2):
                        t = 2 * q + sub
                        acc = psA.tile([P, NW], F32, tag="acc")
                        for k in range(C):
                            ch = (
                                sub * cA + k
                                if k < cA
                                else 2 * cA + sub * cB + (k - cA)
                            )
                            nc.tensor.matmul(
                                acc[:],
                                lhsT=mt[:, ch, :],
                                rhs=y[:, ch, 0:NW],
                                start=(k == 0),
                                stop=(k == C - 1),
                            )
                        nm = wp.tile([P, NW], BF, tag="nm")
                        nc.vector.tensor_copy(out=nm[:], in_=acc[:])
                        if write_bounce:
                            nm8 = w8p.tile([P, NW], F8, tag="nm8")
                            nc.scalar.activation(
                                nm8[:], acc[:],
                                mybir.ActivationFunctionType.Copy,
                            )
                            nc.sync.dma_start(
                                out=bounce.ap()[t * P : (t + 1) * P, 0:NW],
                                in_=nm8[:],
                            )
                        nfc = (NW + P - 1) // P
                        for fc in range(nfc):
                            w_fc = min(P, NW - fc * P)
                            pt = psB.tile([P, P], BF, tag="tp")
                            nc.tensor.transpose(
                                pt[:w_fc, :],
                                nm[:, fc * P : fc * P + w_fc],
                                ident[:],
                            )
                            nc.vector.tensor_copy(
                                out=outT[:w_fc, fc, t * P : (t + 1) * P],
                                in_=pt[:w_fc, :],
                            )
                    if ag_hooks and q in ag_hooks:
                        ag_hooks[q]()

            def dense(l_idx, wt, kc, inT0, h_out, last):
                """h_out = relu(Tx0@W'0 + Tx1@W'1 + P2@W'2 + b), feature-major;
                node-major rows -> bounce (fp8) or bounce_pool (bf16, last)."""
                terms = [(inT0, 0), (t1T, 1), (p2T, 2)]
                for t in range(NT):
                    if last:
                        nmt = wp.tile([P, 256], BF, tag="nmp")
                    else:
                        nmt = w8p.tile([P, 256], F8, tag="nm8d")
                    for fo in range(2):
                        pd = psC.tile([P, P], F32, tag="pd")
                        n_mm = len(terms) * kc
                        i_mm = 0
                        for inT, term in terms:
                            for ki in range(kc):
                                nc.tensor.matmul(
                                    pd[:],
                                    lhsT=w_sb[wt][:, term, ki, fo * P : (fo + 1) * P],
                                    rhs=inT[:, ki, t * P : (t + 1) * P],
                                    start=(i_mm == 0),
                                    stop=(i_mm == n_mm - 1),
                                )
                                i_mm += 1
                        nc.scalar.activation(
                            h_out[:, fo, t * P : (t + 1) * P],
                            pd[:],
                            mybir.ActivationFunctionType.Relu,
                            bias=bvec_sb[:, l_idx, fo : fo + 1],
                        )
                        pt = psB.tile([P, P], BF, tag="tp")
                        nc.tensor.transpose(
                            pt[:], h_out[:, fo, t * P : (t + 1) * P], ident[:]
                        )
                        nc.vector.tensor_copy(
                            out=nmt[:, fo * P : (fo + 1) * P], in_=pt[:]
                        )
                    dst = bounce_pool if last else bounce
                    nc.sync.dma_start(
                        out=dst.ap()[t * P : (t + 1) * P, :], in_=nmt[:]
                    )
                    if not last:
                        if t == TA - 1:
                            allgather_half(None, hgA, 0)
                        elif t == NT - 1:
                            allgather_half(None, hgB, 1)

            # ================= layer 1 (input x, width 64->200) =================
            t1_hooks = {
                (TA // 2) - 1: lambda: allgather_half(None, t1gA, 0),
                NSTEP - 1: lambda: allgather_half(None, t1gB, 1),
            }
            prop(d_xgA, d_xgB, FIN, t1T, True, t1_hooks)
            prop(t1gA, t1gB, FIN, p2T, False)
            dense(0, "w1", 1, hT[0], hT[1], False)

            # ================= layers 2..4 =================
            cur = 1
            for li, wt in ((1, "w2"), (2, "w3"), (3, "w4")):
                prop(hgA, hgB, H, t1T, True, t1_hooks)
                prop(t1gA, t1gB, H, p2T, False)
                dense(li, wt, 2, hT[cur], hT[1 - cur], li == 3)
                cur = 1 - cur

            # ================= pooling + fc + log_softmax =================
            red = {}
            for typ in range(2):  # 0: mean(sum), 1: max
                rT = bigp.tile([P, 2, GPC_OUT], F32, tag=f"red{typ}")
                red[typ] = rT
                for call in range(n_calls):
                    py = yp.tile([P, 2, POOL_GPC * GW], BF, tag="poolY")
                    base = (typ * n_calls + call) * (POOL_GPC * GW // 16)
                    nc.gpsimd.dma_gather(
                        out_ap=py[:],
                        in_ap=bounce_pool.ap(),
                        idxs_ap=pidx_sb[:, base : base + POOL_GPC * GW // 16],
                        num_idxs=POOL_GPC * GW,
                        num_idxs_reg=POOL_GPC * GW,
                        elem_size=256,
                        transpose=True,
                        single_packet=False,
                    )
                    for gg in range(POOL_GPC):
                        gcol = call * POOL_GPC + gg
                        for fc in range(2):
                            nc.vector.tensor_reduce(
                                out=rT[:, fc, gcol : gcol + 1],
                                in_=py[:, fc, gg * GW : (gg + 1) * GW],
                                axis=mybir.AxisListType.X,
                                op=mybir.AluOpType.add
                                if typ == 0
                                else mybir.AluOpType.max,
                            )
            pool_bf = bigp.tile([P, 4, GPC_OUT], BF, tag="poolbf")
            for fc in range(2):
                nc.vector.tensor_tensor(
                    out=pool_bf[:, fc, :],
                    in0=red[0][:, fc, :],
                    in1=cntr_sb[:, :],
                    op=mybir.AluOpType.mult,
                )
                nc.vector.tensor_copy(out=pool_bf[:, 2 + fc, :], in_=red[1][:, fc, :])
            pfc = psD.tile([GPC_OUT, 2], F32, tag="pfc")
            for j in range(4):
                nc.tensor.matmul(
                    pfc[:],
                    lhsT=pool_bf[:, j, :],
                    rhs=fcw_sb[:, j, :],
                    start=(j == 0),
                    stop=(j == 3),
                )
            z = wp.tile([GPC_OUT, 2], F32, tag="z")
            nc.vector.tensor_tensor(
                out=z[:], in0=pfc[:], in1=fcb_sb[:, :], op=mybir.AluOpType.add
            )
            mx = wp.tile([GPC_OUT, 1], F32, tag="mx")
            nc.vector.tensor_reduce(
                out=mx[:], in_=z[:], axis=mybir.AxisListType.X, op=mybir.AluOpType.max
            )
            zm = wp.tile([GPC_OUT, 2], F32, tag="zm")
            nc.vector.tensor_scalar(
                zm[:], z[:], mx[:], None, mybir.AluOpType.subtract
            )
            ez = wp.tile([GPC_OUT, 2], F32, tag="ez")
            nc.scalar.activation(ez[:], zm[:], mybir.ActivationFunctionType.Exp)
            sz = wp.tile([GPC_OUT, 1], F32, tag="sz")
            nc.vector.tensor_reduce(
                out=sz[:], in_=ez[:], axis=mybir.AxisListType.X, op=mybir.AluOpType.add
            )
            lz = wp.tile([GPC_OUT, 1], F32, tag="lz")
            nc.scalar.activation(lz[:], sz[:], mybir.ActivationFunctionType.Ln)
            oz = wp.tile([GPC_OUT, 2], F32, tag="oz")
            nc.vector.tensor_scalar(
                oz[:], zm[:], lz[:], None, mybir.AluOpType.subtract
            )
            nc.sync.dma_start(out=d_out.ap(), in_=oz[:])

    nc.finalize()
    return nc


def kernel(**inputs):
    x = np.asarray(inputs["x"], np.float32)
    edge_index = np.asarray(inputs["edge_index"])
    batch = np.asarray(inputs["batch"])
    lmax = np.asarray(inputs["lmax"], np.float32)

    pp = _preprocess(x, edge_index, batch, lmax)
    wts = _pack_weights(
        np.asarray(inputs["W1"], np.float32), np.asarray(inputs["W2"], np.float32),
        np.asarray(inputs["W3"], np.float32), np.asarray(inputs["W4"], np.float32),
        np.asarray(inputs["b1"], np.float32), np.asarray(inputs["b2"], np.float32),
        np.asarray(inputs["b3"], np.float32), np.asarray(inputs["b4"], np.float32),
        np.asarray(inputs["fc_w"], np.float32), np.asarray(inputs["fc_b"], np.float32),
    )

    key = (
        pp["NL"], pp["NT"], pp["NSTEP"], pp["C"], pp["cA"], pp["cB"],
        pp["GW"], pp["TA"], pp["HLA"], pp["HLB"],
    )
    if key not in _cache:
        _cache[key] = _build(*key)
    nc = _cache[key]

    shared = dict(
        xgA=pp["xgA"], xgB=pp["xgB"],
        w1=wts["w1"], w2=wts["w2"], w3=wts["w3"], w4=wts["w4"],
        bvec=wts["bvec"], fcw=wts["fcw"], fcb=wts["fcb"],
    )
    in_maps = [
        dict(
            shared,
            xloc=pp["xloc"][c], idx=pp["idx"][c], mm=pp["mm"][c],
            pidx=pp["pidx"][c], cntr=pp["cntr"][c],
        )
        for c in range(NCORES)
    ]
    trace = bool(int(__import__("os").environ.get("KERNEL_TRACE", "0")))
    res = run_bass_kernel_spmd(nc, in_maps, list(range(NCORES)), trace=trace)
    if trace:
        kernel.last_exec_time_ns = res.exec_time_ns
        kernel.last_results = res
    out = np.concatenate([res.results[c]["out"] for c in range(NCORES)], axis=0)
    return out.astype(np.float32)


kernel.last_exec_time_ns = None


# revision 3
# speedup vs baseline: 1.5368x; 1.4548x over previous
"""ChebNet (K=3, 4 layers, H=200) on 8 TRN2 NeuronCores.

v3: gathers are row-rate-limited (~8-9ns/row per SWDGE queue stream), so
the prop issues 4 concurrent dma_gather calls per 2-tile step (queues
0-3).  Self-edges are removed from the gather and handled as sequential
32KB loads of the local node-major input plus a diagonal selection
chunk.  fp8(e4m3) tables/selection matrices keep the mm stream and the
7 single AllGathers small.  Chebyshev refactor:
  out = Tx0@(W0-W2) + Tx1@W1 + (L Tx1)@(2 W2) + b.
"""

import sys
import types

sys.path.insert(0, "/opt/trn_rl_repo")

import ml_dtypes
import numpy as np

# antenv.axon_hooks shim (lets run_bass_kernel_spmd(trace=True) profile)
try:
    import trn_agent_boot.trn_boot as _tb

    if "antenv.axon_hooks" not in sys.modules:
        _hook = _tb._ntff_profile_via_ctypes("/opt/axon/libaxon_pjrt.so")
        _m = types.ModuleType("antenv.axon_hooks")
        _m.get_axon_ntff_profile_hook = lambda: _hook
        _m.set_axon_ntff_profile_hook = lambda h: None
        sys.modules["antenv.axon_hooks"] = _m
except Exception:
    pass

import concourse.bass as bass
import concourse.mybir as mybir
import concourse.tile as tile
from concourse import bacc
from concourse.bass_utils import run_bass_kernel_spmd
from concourse.masks import make_identity

BF16 = ml_dtypes.bfloat16
FP8 = ml_dtypes.float8_e4m3  # TRN float8e4
NCORES = 8
G = 256
GPC_OUT = G // NCORES  # graphs per core = 32
H = 200
FIN = 64
POOL_GPC = 8
P = 128

_cache = {}


def _wrap_idx(vals):
    """[n] int -> [128, n//16] int16 in dma_gather wrapped+replicated layout."""
    n = len(vals)
    assert n % 16 == 0
    w16 = np.asarray(vals, np.int16).reshape(n // 16, 16).T
    return np.tile(w16, (8, 1))


def _preprocess(x, edge_index, batch, lmax):
    N = x.shape[0]
    src = edge_index[0].astype(np.int64)
    dst = edge_index[1].astype(np.int64)
    batch = batch.astype(np.int64)

    deg = np.bincount(src, minlength=N).astype(np.float32)
    dis = np.where(deg > 0, np.maximum(deg, 1.0) ** -0.5, 0.0).astype(np.float32)
    scale = (2.0 / lmax).astype(np.float32)
    w_edge = (-dis[src] * dis[dst] * scale[batch[src]]).astype(np.float32)
    diag = (scale[batch] - 1.0).astype(np.float32)  # [N], self-term weights

    node_core = (batch // GPC_OUT).astype(np.int64)
    counts = np.bincount(node_core, minlength=NCORES)
    assert counts.min() > 0
    NL = int(np.ceil((counts.max() + 1) / P) * P)
    NT = NL // P
    assert NT % 2 == 0
    NSTEP = NT // 2
    assert 4 * NL < 32768
    core_start = np.zeros(NCORES + 1, np.int64)
    core_start[1:] = np.cumsum(counts)
    slot = np.arange(N) - core_start[node_core]

    # real edges only (self/diag handled separately)
    e_core = node_core[dst]
    e_tile = (slot[dst] >> 7).astype(np.int64)
    e_dl = (slot[dst] & 127).astype(np.int64)
    e_half = (node_core[src] >= 4).astype(np.int64)  # table A: cores 0-3
    e_grow = np.where(e_half == 0, node_core[src] * NL + slot[src],
                      (node_core[src] - 4) * NL + slot[src])

    order = np.lexsort((e_half, e_tile, e_core))
    gkey = ((e_core * NT + e_tile) * 2 + e_half)[order]
    grp_start_mask = np.ones(len(gkey), bool)
    grp_start_mask[1:] = gkey[1:] != gkey[:-1]
    grp_idx = np.flatnonzero(grp_start_mask)
    within = np.arange(len(gkey)) - np.repeat(
        grp_idx, np.diff(np.append(grp_idx, len(gkey)))
    )
    cnts = np.zeros(NCORES * NT * 2, np.int64)
    uk, uc = np.unique(gkey, return_counts=True)
    cnts[uk] = uc
    cA = int(np.ceil(cnts.reshape(-1, 2)[:, 0].max() / P))
    cB = int(np.ceil(cnts.reshape(-1, 2)[:, 1].max() / P))
    C = cA + cB
    NCH = 2 * C + 2  # per-step chunks: A(2q) A(2q+1) B(2q) B(2q+1) self self

    # per-step chunk position of each edge
    t_of_e = gkey // 2 % NT
    sub_of_e = t_of_e % 2
    q_of_e = t_of_e // 2
    h_of_e = gkey % 2
    chunk = np.where(
        h_of_e == 0,
        sub_of_e * cA + within // P,
        2 * cA + sub_of_e * cB + within // P,
    )
    pos = (q_of_e * NCH + chunk) * P + within % P  # mm slot (incl self chunks)
    gpos = (q_of_e * 2 * C + chunk) * P + within % P  # idx slot (gathered only)
    ecore_sorted = e_core[order]

    idx_arr = np.zeros((NCORES, NSTEP * 2 * C * P), np.int64)
    idx_arr[ecore_sorted, gpos] = e_grow[order]
    assert idx_arr.max() < 32768
    mm_arr = np.zeros((NCORES, NSTEP * NCH * P), np.float32)
    dl_arr = np.zeros((NCORES, NSTEP * NCH * P), np.int64)
    mm_arr[ecore_sorted, pos] = w_edge[order]
    dl_arr[ecore_sorted, pos] = e_dl[order]

    idx_dev, mm_dev = [], []
    for c in range(NCORES):
        a3 = idx_arr[c].reshape(NSTEP, 2 * C, P)
        cols = []
        for q in range(NSTEP):
            cols.append(_wrap_idx(a3[q, 0:cA].reshape(-1)))
            cols.append(_wrap_idx(a3[q, cA : 2 * cA].reshape(-1)))
            cols.append(_wrap_idx(a3[q, 2 * cA : 2 * cA + cB].reshape(-1)))
            cols.append(_wrap_idx(a3[q, 2 * cA + cB :].reshape(-1)))
        idx_dev.append(np.hstack(cols))  # [128, NSTEP*2C*8]
        mm = np.zeros((P, NSTEP * NCH * P), FP8)
        kk = np.arange(NSTEP * NCH * P)
        mm[kk % P, (kk // P) * P + dl_arr[c]] = mm_arr[c].astype(FP8)
        # self (diag) chunks: mm[e, (q*NCH + 2C + sub)*P + e] = diag[node]
        nodes = np.arange(counts[c]) + core_start[c]
        s = slot[nodes]
        t_of = s >> 7
        e_of = s & 127
        q_of = t_of // 2
        sub_of = t_of % 2
        mm[e_of, (q_of * NCH + 2 * C + sub_of) * P + e_of] = diag[nodes].astype(FP8)
        mm_dev.append(mm)

    # x tables (fp8, 256-wide rows, 64 used)
    xg = np.zeros((NCORES * NL, 256), FP8)
    xg[node_core * NL + slot, :FIN] = x.astype(FP8)
    xgA = np.ascontiguousarray(xg[: 4 * NL])
    xgB = np.ascontiguousarray(xg[4 * NL :])
    xloc8 = [
        np.ascontiguousarray(xg[c * NL : (c + 1) * NL]) for c in range(NCORES)
    ]
    xg_nm = np.zeros((NCORES * NL, P), BF16)
    xg_nm[node_core * NL + slot, :FIN] = x.astype(BF16)
    xloc = [np.ascontiguousarray(xg_nm[c * NL : (c + 1) * NL]) for c in range(NCORES)]

    # pooling windows (bf16 bounce_pool, local rows)
    gcnt = np.bincount(batch, minlength=G).astype(np.int64)
    assert gcnt.min() > 0
    GW = int(np.ceil(gcnt.max() / 16) * 16)
    n_calls = GPC_OUT // POOL_GPC
    pidx = []
    cntr = []
    for c in range(NCORES):
        zrow = int(counts[c])
        mean_cols, max_cols = [], []
        for call in range(n_calls):
            mvals = np.zeros(POOL_GPC * GW, np.int64)
            xvals = np.zeros(POOL_GPC * GW, np.int64)
            for gg in range(POOL_GPC):
                g_id = c * GPC_OUT + call * POOL_GPC + gg
                lo = np.searchsorted(batch, g_id, "left")
                hi = np.searchsorted(batch, g_id, "right")
                rows = slot[lo:hi]
                k = hi - lo
                mvals[gg * GW : gg * GW + k] = rows
                mvals[gg * GW + k : (gg + 1) * GW] = zrow
                xvals[gg * GW : gg * GW + k] = rows
                xvals[gg * GW + k : (gg + 1) * GW] = rows[0]
            mean_cols.append(_wrap_idx(mvals))
            max_cols.append(_wrap_idx(xvals))
        pidx.append(np.hstack(mean_cols + max_cols))
        cr = (1.0 / np.maximum(gcnt[c * GPC_OUT : (c + 1) * GPC_OUT], 1.0)).astype(
            np.float32
        )
        cntr.append(np.tile(cr.reshape(1, GPC_OUT), (P, 1)))

    return dict(
        NL=NL, NT=NT, NSTEP=NSTEP, C=C, cA=cA, cB=cB, NCH=NCH, GW=GW,
        idx=idx_dev, mm=mm_dev, xgA=xgA, xgB=xgB, xloc=xloc, xloc8=xloc8,
        pidx=pidx, cntr=cntr,
    )


def _pack_weights(W1, W2, W3, W4, b1, b2, b3, b4, fc_w, fc_b):
    def cheb_pack(W, kin_chunks):
        Wp = np.stack([W[0] - W[2], W[1], 2.0 * W[2]]).astype(np.float32)
        out = np.zeros((3, kin_chunks, P, 256), np.float32)
        fin = W.shape[1]
        for ki in range(kin_chunks):
            lo = ki * P
            hi = min(fin, lo + P)
            if hi > lo:
                out[:, ki, : hi - lo, :H] = Wp[:, lo:hi, :]
        return out.astype(BF16)

    w1 = cheb_pack(W1, 1)
    w2 = cheb_pack(W2, 2)
    w3 = cheb_pack(W3, 2)
    w4 = cheb_pack(W4, 2)
    bvec = np.zeros((P, 4, 2), np.float32)
    for li, b in enumerate([b1, b2, b3, b4]):
        for fo in range(2):
            seg = b[fo * P : min(H, (fo + 1) * P)]
            bvec[: len(seg), li, fo] = seg
    fcw = np.zeros((P, 4, 2), np.float32)
    fcw[:, 0] = fc_w[0:P]
    fcw[: H - P, 1] = fc_w[P:H]
    fcw[:, 2] = fc_w[H : H + P]
    fcw[: H - P, 3] = fc_w[H + P : 2 * H]
    fcb = np.tile(fc_b.astype(np.float32).reshape(1, 2), (GPC_OUT, 1))
    return dict(
        w1=w1, w2=w2, w3=w3, w4=w4, bvec=bvec, fcw=fcw.astype(BF16), fcb=fcb
    )


def _build(NL, NT, NSTEP, C, cA, cB, NCH, GW):
    F32, BF, F8, I16 = (
        mybir.dt.float32, mybir.dt.bfloat16, mybir.dt.float8e4, mybir.dt.int16
    )
    nc = bacc.Bacc(None, num_devices=NCORES, num_swdge_queues=4)
    rg = [list(range(NCORES))]
    n_calls = GPC_OUT // POOL_GPC

    d_xgA = nc.declare_dram_parameter("xgA", [4 * NL, 256], F8, isOutput=False)
    d_xgB = nc.declare_dram_parameter("xgB", [4 * NL, 256], F8, isOutput=False)
    d_xloc8 = nc.declare_dram_parameter("xloc8", [NL, 256], F8, isOutput=False)
    d_xloc = nc.declare_dram_parameter("xloc", [NL, P], BF, isOutput=False)
    d_idx = nc.declare_dram_parameter("idx", [P, NSTEP * 2 * C * 8], I16, isOutput=False)
    d_mm = nc.declare_dram_parameter("mm", [P, NSTEP * NCH * P], F8, isOutput=False)
    d_pidx = nc.declare_dram_parameter(
        "pidx", [P, 2 * n_calls * POOL_GPC * GW // 16], I16, isOutput=False
    )
    d_cntr = nc.declare_dram_parameter("cntr", [P, GPC_OUT], F32, isOutput=False)
    d_w1 = nc.declare_dram_parameter("w1", [3, 1, P, 256], BF, isOutput=False)
    d_w2 = nc.declare_dram_parameter("w2", [3, 2, P, 256], BF, isOutput=False)
    d_w3 = nc.declare_dram_parameter("w3", [3, 2, P, 256], BF, isOutput=False)
    d_w4 = nc.declare_dram_parameter("w4", [3, 2, P, 256], BF, isOutput=False)
    d_bvec = nc.declare_dram_parameter("bvec", [P, 4, 2], F32, isOutput=False)
    d_fcw = nc.declare_dram_parameter("fcw", [P, 4, 2], BF, isOutput=False)
    d_fcb = nc.declare_dram_parameter("fcb", [GPC_OUT, 2], F32, isOutput=False)
    d_out = nc.declare_dram_parameter("out", [GPC_OUT, 2], F32, isOutput=True)

    bounce = nc.dram_tensor("bounce", [NL, 256], F8)  # h (dense out, fp8)
    t1b = nc.dram_tensor("t1b", [NL, 256], F8)  # t1 (prop1 out, fp8)
    bounce_pool = nc.dram_tensor("bounce_pool", [NL, 256], BF)  # last h
    hg = nc.dram_tensor("hg", [NCORES * NL, 256], F8, addr_space="Shared")
    t1g = nc.dram_tensor("t1g", [NCORES * NL, 256], F8, addr_space="Shared")

    with tile.TileContext(nc) as tc:
        with (
            tc.tile_pool(name="const", bufs=1) as cp,
            tc.tile_pool(name="big", bufs=1) as bigp,
            tc.tile_pool(name="work", bufs=3) as wp,
            tc.tile_pool(name="w8", bufs=3) as w8p,
            tc.tile_pool(name="ypool", bufs=2) as yp,
            tc.tile_pool(name="ygath", bufs=3) as ygp,
            tc.tile_pool(name="mpool", bufs=3) as mp,
            tc.tile_pool(name="psA", bufs=3, space="PSUM") as psA,
            tc.tile_pool(name="psB", bufs=2, space="PSUM") as psB,
            tc.tile_pool(name="psC", bufs=2, space="PSUM") as psC,
            tc.tile_pool(name="psD", bufs=1, space="PSUM") as psD,
        ):
            idx_sb = cp.tile([P, NSTEP * 2 * C * 8], I16)
            nc.sync.dma_start(out=idx_sb[:], in_=d_idx.ap())
            pidx_sb = cp.tile([P, 2 * n_calls * POOL_GPC * GW // 16], I16)
            nc.sync.dma_start(out=pidx_sb[:], in_=d_pidx.ap())
            w_sb = {}
            for nm_, dp, kc in (
                ("w1", d_w1, 1), ("w2", d_w2, 2), ("w3", d_w3, 2), ("w4", d_w4, 2)
            ):
                t = cp.tile([P, 3, kc, 256], BF, tag=nm_)
                nc.sync.dma_start(
                    out=t[:], in_=dp.ap().rearrange("t k p f -> p t k f")
                )
                w_sb[nm_] = t
            bvec_sb = cp.tile([P, 4, 2], F32)
            nc.sync.dma_start(out=bvec_sb[:], in_=d_bvec.ap())
            fcw_sb = cp.tile([P, 4, 2], BF)
            nc.sync.dma_start(out=fcw_sb[:], in_=d_fcw.ap())
            fcb_sb = cp.tile([GPC_OUT, 2], F32)
            nc.sync.dma_start(out=fcb_sb[:], in_=d_fcb.ap())
            cntr_sb = cp.tile([P, GPC_OUT], F32)
            nc.sync.dma_start(out=cntr_sb[:], in_=d_cntr.ap())

            ident = cp.tile([P, P], BF)
            make_identity(nc, ident[:])

            hT = [
                bigp.tile([P, 2, NT * P], BF, tag=f"hT{i}", name=f"hT{i}")
                for i in range(2)
            ]
            t1T = bigp.tile([P, 2, NT * P], BF, tag="t1T")
            p2T = bigp.tile([P, 2, NT * P], BF, tag="p2T")
            for buf in (hT[0], hT[1], t1T, p2T):
                nc.vector.memset(buf[:], 0.0)
            zt = cp.tile([P, 256], BF)
            nc.vector.memset(zt[:], 0.0)
            for t in range(NT):
                nc.sync.dma_start(
                    out=bounce_pool.ap()[t * P : (t + 1) * P, :], in_=zt[:]
                )

            for t in range(NT):
                xt = wp.tile([P, P], BF, tag="xload")
                nc.sync.dma_start(out=xt[:], in_=d_xloc.ap()[t * P : (t + 1) * P, :])
                pt = psB.tile([P, P], BF, tag="tp")
                nc.tensor.transpose(pt[:], xt[:], ident[:])
                nc.vector.tensor_copy(out=hT[0][:, 0, t * P : (t + 1) * P], in_=pt[:])

            def allgather(src, dst):
                nc.gpsimd.collective_compute(
                    "AllGather",
                    mybir.AluOpType.bypass,
                    replica_groups=rg,
                    ins=[src.ap().opt()],
                    outs=[dst.ap().opt()],
                )

            def prop(tabA_ap, tabB_ap, self_dram, NW, outT, write_t1):
                # gather calls: (chunk range, idx col offset units of 8/chunk)
                segs = [
                    (0, cA), (cA, 2 * cA), (2 * cA, 2 * cA + cB), (2 * cA + cB, 2 * C)
                ]
                for q in range(NSTEP):
                    y = ygp.tile([P, NCH, 256], F8, tag="Y")
                    for j, (lo, hi) in enumerate(segs):
                        nc.gpsimd.dma_gather(
                            out_ap=y[:, lo:hi, :],
                            in_ap=tabA_ap if j < 2 else tabB_ap,
                            idxs_ap=idx_sb[
                                :, q * 2 * C * 8 + lo * 8 : q * 2 * C * 8 + hi * 8
                            ],
                            num_idxs=(hi - lo) * P,
                            num_idxs_reg=(hi - lo) * P,
                            elem_size=256,
                            single_packet=False,
                            queue_num=(q + j) % 4,
                        )
                    for sub in range(2):
                        t = 2 * q + sub
                        nc.sync.dma_start(
                            out=y[:, 2 * C + sub, :],
                            in_=self_dram.ap()[t * P : (t + 1) * P, :],
                        )
                    mt = mp.tile([P, NCH, P], F8, tag="mt")
                    nc.sync.dma_start(
                        out=mt[:],
                        in_=d_mm.ap()[:, q * NCH * P : (q + 1) * NCH * P].rearrange(
                            "e (k d) -> e k d", d=P
                        ),
                    )
                    for sub in range(2):
                        t = 2 * q + sub
                        chunks = (
                            list(range(sub * cA, (sub + 1) * cA))
                            + list(range(2 * cA + sub * cB, 2 * cA + (sub + 1) * cB))
                            + [2 * C + sub]
                        )
                        acc = psA.tile([P, NW], F32, tag="acc")
                        for ki, ch in enumerate(chunks):
                            nc.tensor.matmul(
                                acc[:],
                                lhsT=mt[:, ch, :],
                                rhs=y[:, ch, 0:NW],
                                start=(ki == 0),
                                stop=(ki == len(chunks) - 1),
                            )
                        nm = wp.tile([P, NW], BF, tag="nm")
                        nc.vector.tensor_copy(out=nm[:], in_=acc[:])
                        if write_t1:
                            nm8 = w8p.tile([P, NW], F8, tag="nm8")
                            nc.scalar.activation(
                                nm8[:], acc[:], mybir.ActivationFunctionType.Copy
                            )
                            nc.sync.dma_start(
                                out=t1b.ap()[t * P : (t + 1) * P, 0:NW], in_=nm8[:]
                            )
                        nfc = (NW + P - 1) // P
                        for fc in range(nfc):
                            w_fc = min(P, NW - fc * P)
                            pt = psB.tile([P, P], BF, tag="tp")
                            nc.tensor.transpose(
                                pt[:w_fc, :],
                                nm[:, fc * P : fc * P + w_fc],
                                ident[:],
                            )
                            nc.vector.tensor_copy(
                                out=outT[:w_fc, fc, t * P : (t + 1) * P],
                                in_=pt[:w_fc, :],
                            )

            def dense(l_idx, wt, kc, inT0, h_out, last):
                terms = [(inT0, 0), (t1T, 1), (p2T, 2)]
                for t in range(NT):
                    if last:
                        nmt = wp.tile([P, 256], BF, tag="nmp")
                    else:
                        nmt = w8p.tile([P, 256], F8, tag="nm8d")
                    for fo in range(2):
                        pd = psC.tile([P, P], F32, tag="pd")
                        n_mm = len(terms) * kc
                        i_mm = 0
                        for inT, term in terms:
                            for ki in range(kc):
                                nc.tensor.matmul(
                                    pd[:],
                                    lhsT=w_sb[wt][:, term, ki, fo * P : (fo + 1) * P],
                                    rhs=inT[:, ki, t * P : (t + 1) * P],
                                    start=(i_mm == 0),
                                    stop=(i_mm == n_mm - 1),
                                )
                                i_mm += 1
                        nc.scalar.activation(
                            h_out[:, fo, t * P : (t + 1) * P],
                            pd[:],
                            mybir.ActivationFunctionType.Relu,
                            bias=bvec_sb[:, l_idx, fo : fo + 1],
                        )
                        pt = psB.tile([P, P], BF, tag="tp")
                        nc.tensor.transpose(
                            pt[:], h_out[:, fo, t * P : (t + 1) * P], ident[:]
                        )
                        if last:
                            nc.vector.tensor_copy(
                                out=nmt[:, fo * P : (fo + 1) * P], in_=pt[:]
                            )
                        else:
                            nc.scalar.activation(
                                nmt[:, fo * P : (fo + 1) * P], pt[:],
                                mybir.ActivationFunctionType.Copy,
                            )
                    dst = bounce_pool if last else bounce
                    nc.sync.dma_start(
                        out=dst.ap()[t * P : (t + 1) * P, :], in_=nmt[:]
                    )

            # ================= layer 1 (input x, width 64->200) =================
            prop(d_xgA.ap(), d_xgB.ap(), d_xloc8, FIN, t1T, True)
            allgather(t1b, t1g)
            prop(t1g.ap()[0 : 4 * NL, :], t1g.ap()[4 * NL :, :], t1b, FIN, p2T, False)
            dense(0, "w1", 1, hT[0], hT[1], False)
            allgather(bounce, hg)

            # ================= layers 2..4 =================
            cur = 1
            for li, wt in ((1, "w2"), (2, "w3"), (3, "w4")):
                prop(hg.ap()[0 : 4 * NL, :], hg.ap()[4 * NL :, :], bounce, H, t1T, True)
                allgather(t1b, t1g)
                prop(
                    t1g.ap()[0 : 4 * NL, :], t1g.ap()[4 * NL :, :], t1b, H, p2T, False
                )
                dense(li, wt, 2, hT[cur], hT[1 - cur], li == 3)
                cur = 1 - cur
                if li < 3:
                    allgather(bounce, hg)

            # ================= pooling + fc + log_softmax =================
            red = {}
            for typ in range(2):
                rT = bigp.tile([P, 2, GPC_OUT], F32, tag=f"red{typ}")
                red[typ] = rT
                for call in range(n_calls):
                    py = yp.tile([P, 2, POOL_GPC * GW], BF, tag="poolY")
                    base = (typ * n_calls + call) * (POOL_GPC * GW // 16)
                    nc.gpsimd.dma_gather(
                        out_ap=py[:],
                        in_ap=bounce_pool.ap(),
                        idxs_ap=pidx_sb[:, base : base + POOL_GPC * GW // 16],
                        num_idxs=POOL_GPC * GW,
                        num_idxs_reg=POOL_GPC * GW,
                        elem_size=256,
                        transpose=True,
                        single_packet=False,
                    )
                    for gg in range(POOL_GPC):
                        gcol = call * POOL_GPC + gg
                        for fc in range(2):
                            nc.vector.tensor_reduce(
                                out=rT[:, fc, gcol : gcol + 1],
                                in_=py[:, fc, gg * GW : (gg + 1) * GW],
                                axis=mybir.AxisListType.X,
                                op=mybir.AluOpType.add
                                if typ == 0
                                else mybir.AluOpType.max,
                            )
            pool_bf = bigp.tile([P, 4, GPC_OUT], BF, tag="poolbf")
            for fc in range(2):
                nc.vector.tensor_tensor(
                    out=pool_bf[:, fc, :],
                    in0=red[0][:, fc, :],
                    in1=cntr_sb[:, :],
                    op=mybir.AluOpType.mult,
                )
                nc.vector.tensor_copy(out=pool_bf[:, 2 + fc, :], in_=red[1][:, fc, :])
            pfc = psD.tile([GPC_OUT, 2], F32, tag="pfc")
            for j in range(4):
                nc.tensor.matmul(
                    pfc[:],
                    lhsT=pool_bf[:, j, :],
                    rhs=fcw_sb[:, j, :],
                    start=(j == 0),
                    stop=(j == 3),
                )
            z = wp.tile([GPC_OUT, 2], F32, tag="z")
            nc.vector.tensor_tensor(
                out=z[:], in0=pfc[:], in1=fcb_sb[:, :], op=mybir.AluOpType.add
            )
            mx = wp.tile([GPC_OUT, 1], F32, tag="mx")
            nc.vector.tensor_reduce(
                out=mx[:], in_=z[:], axis=mybir.AxisListType.X, op=mybir.AluOpType.max
            )
            zm = wp.tile([GPC_OUT, 2], F32, tag="zm")
            nc.vector.tensor_scalar(
                zm[:], z[:], mx[:], None, mybir.AluOpType.subtract
            )
            ez = wp.tile([GPC_OUT, 2], F32, tag="ez")
            nc.scalar.activation(ez[:], zm[:], mybir.ActivationFunctionType.Exp)
            sz = wp.tile([GPC_OUT, 1], F32, tag="sz")
            nc.vector.tensor_reduce(
                out=sz[:], in_=ez[:], axis=mybir.AxisListType.X, op=mybir.AluOpType.add
            )
            lz = wp.tile([GPC_OUT, 1], F32, tag="lz")
            nc.scalar.activation(lz[:], sz[:], mybir.ActivationFunctionType.Ln)
            oz = wp.tile([GPC_OUT, 2], F32, tag="oz")
            nc.vector.tensor_scalar(
                oz[:], zm[:], lz[:], None, mybir.AluOpType.subtract
            )
            nc.sync.dma_start(out=d_out.ap(), in_=oz[:])

    nc.finalize()
    return nc


def kernel(**inputs):
    x = np.asarray(inputs["x"], np.float32)
    edge_index = np.asarray(inputs["edge_index"])
    batch = np.asarray(inputs["batch"])
    lmax = np.asarray(inputs["lmax"], np.float32)

    pp = _preprocess(x, edge_index, batch, lmax)
    wts = _pack_weights(
        np.asarray(inputs["W1"], np.float32), np.asarray(inputs["W2"], np.float32),
        np.asarray(inputs["W3"], np.float32), np.asarray(inputs["W4"], np.float32),
        np.asarray(inputs["b1"], np.float32), np.asarray(inputs["b2"], np.float32),
        np.asarray(inputs["b3"], np.float32), np.asarray(inputs["b4"], np.float32),
        np.asarray(inputs["fc_w"], np.float32), np.asarray(inputs["fc_b"], np.float32),
    )

    key = (
        pp["NL"], pp["NT"], pp["NSTEP"], pp["C"], pp["cA"], pp["cB"],
        pp["NCH"], pp["GW"],
    )
    if key not in _cache:
        _cache[key] = _build(*key)
    nc = _cache[key]

    shared = dict(
        xgA=pp["xgA"], xgB=pp["xgB"],
        w1=wts["w1"], w2=wts["w2"], w3=wts["w3"], w4=wts["w4"],
        bvec=wts["bvec"], fcw=wts["fcw"], fcb=wts["fcb"],
    )
    in_maps = [
        dict(
            shared,
            xloc=pp["xloc"][c], xloc8=pp["xloc8"][c],
            idx=pp["idx"][c], mm=pp["mm"][c],
            pidx=pp["pidx"][c], cntr=pp["cntr"][c],
        )
        for c in range(NCORES)
    ]
    trace = bool(int(__import__("os").environ.get("KERNEL_TRACE", "0")))
    res = run_bass_kernel_spmd(nc, in_maps, list(range(NCORES)), trace=trace)
    if trace:
        kernel.last_exec_time_ns = res.exec_time_ns
        kernel.last_results = res
    out = np.concatenate([res.results[c]["out"] for c in range(NCORES)], axis=0)
    return out.astype(np.float32)


kernel.last_exec_time_ns = None


# revision 4
# speedup vs baseline: 1.5533x; 1.0107x over previous
"""ChebNet (K=3, 4 layers, H=200) on 8 TRN2 NeuronCores.

v3: gathers are row-rate-limited (~8-9ns/row per SWDGE queue stream), so
the prop issues 4 concurrent dma_gather calls per 2-tile step (queues
0-3).  Self-edges are removed from the gather and handled as sequential
32KB loads of the local node-major input plus a diagonal selection
chunk.  fp8(e4m3) tables/selection matrices keep the mm stream and the
7 single AllGathers small.  Chebyshev refactor:
  out = Tx0@(W0-W2) + Tx1@W1 + (L Tx1)@(2 W2) + b.
"""

import sys
import types

sys.path.insert(0, "/opt/trn_rl_repo")

import ml_dtypes
import numpy as np

# antenv.axon_hooks shim (lets run_bass_kernel_spmd(trace=True) profile)
try:
    import trn_agent_boot.trn_boot as _tb

    if "antenv.axon_hooks" not in sys.modules:
        _hook = _tb._ntff_profile_via_ctypes("/opt/axon/libaxon_pjrt.so")
        _m = types.ModuleType("antenv.axon_hooks")
        _m.get_axon_ntff_profile_hook = lambda: _hook
        _m.set_axon_ntff_profile_hook = lambda h: None
        sys.modules["antenv.axon_hooks"] = _m
except Exception:
    pass

import concourse.bass as bass
import concourse.mybir as mybir
import concourse.tile as tile
from concourse import bacc
from concourse.bass_utils import run_bass_kernel_spmd
from concourse.masks import make_identity

BF16 = ml_dtypes.bfloat16
FP8 = ml_dtypes.float8_e4m3  # TRN float8e4
NCORES = 8
G = 256
GPC_OUT = G // NCORES  # graphs per core = 32
H = 200
FIN = 64
POOL_GPC = 8
P = 128

_cache = {}


def _wrap_idx(vals):
    """[n] int -> [128, n//16] int16 in dma_gather wrapped+replicated layout."""
    n = len(vals)
    assert n % 16 == 0
    w16 = np.asarray(vals, np.int16).reshape(n // 16, 16).T
    return np.tile(w16, (8, 1))


def _preprocess(x, edge_index, batch, lmax):
    N = x.shape[0]
    src = edge_index[0].astype(np.int64)
    dst = edge_index[1].astype(np.int64)
    batch = batch.astype(np.int64)

    deg = np.bincount(src, minlength=N).astype(np.float32)
    dis = np.where(deg > 0, np.maximum(deg, 1.0) ** -0.5, 0.0).astype(np.float32)
    scale = (2.0 / lmax).astype(np.float32)
    w_edge = (-dis[src] * dis[dst] * scale[batch[src]]).astype(np.float32)
    diag = (scale[batch] - 1.0).astype(np.float32)  # [N], self-term weights

    node_core = (batch // GPC_OUT).astype(np.int64)
    counts = np.bincount(node_core, minlength=NCORES)
    assert counts.min() > 0
    NL = int(np.ceil((counts.max() + 1) / P) * P)
    NT = NL // P
    assert NT % 2 == 0
    NSTEP = NT // 2
    assert 4 * NL < 32768
    core_start = np.zeros(NCORES + 1, np.int64)
    core_start[1:] = np.cumsum(counts)
    slot = np.arange(N) - core_start[node_core]

    # real edges only (self/diag handled separately)
    e_core = node_core[dst]
    e_tile = (slot[dst] >> 7).astype(np.int64)
    e_dl = (slot[dst] & 127).astype(np.int64)
    e_half = (node_core[src] >= 4).astype(np.int64)  # table A: cores 0-3
    e_grow = np.where(e_half == 0, node_core[src] * NL + slot[src],
                      (node_core[src] - 4) * NL + slot[src])

    order = np.lexsort((e_half, e_tile, e_core))
    gkey = ((e_core * NT + e_tile) * 2 + e_half)[order]
    grp_start_mask = np.ones(len(gkey), bool)
    grp_start_mask[1:] = gkey[1:] != gkey[:-1]
    grp_idx = np.flatnonzero(grp_start_mask)
    within = np.arange(len(gkey)) - np.repeat(
        grp_idx, np.diff(np.append(grp_idx, len(gkey)))
    )
    cnts = np.zeros(NCORES * NT * 2, np.int64)
    uk, uc = np.unique(gkey, return_counts=True)
    cnts[uk] = uc
    cA = int(np.ceil(cnts.reshape(-1, 2)[:, 0].max() / P))
    cB = int(np.ceil(cnts.reshape(-1, 2)[:, 1].max() / P))
    C = cA + cB
    NCH = 2 * C + 2  # per-step chunks: A(2q) A(2q+1) B(2q) B(2q+1) self self

    # per-step chunk position of each edge
    t_of_e = gkey // 2 % NT
    sub_of_e = t_of_e % 2
    q_of_e = t_of_e // 2
    h_of_e = gkey % 2
    chunk = np.where(
        h_of_e == 0,
        sub_of_e * cA + within // P,
        2 * cA + sub_of_e * cB + within // P,
    )
    pos = (q_of_e * NCH + chunk) * P + within % P  # mm slot (incl self chunks)
    gpos = (q_of_e * 2 * C + chunk) * P + within % P  # idx slot (gathered only)
    ecore_sorted = e_core[order]

    idx_arr = np.zeros((NCORES, NSTEP * 2 * C * P), np.int64)
    idx_arr[ecore_sorted, gpos] = e_grow[order]
    assert idx_arr.max() < 32768
    mm_arr = np.zeros((NCORES, NSTEP * NCH * P), np.float32)
    dl_arr = np.zeros((NCORES, NSTEP * NCH * P), np.int64)
    mm_arr[ecore_sorted, pos] = w_edge[order]
    dl_arr[ecore_sorted, pos] = e_dl[order]

    idx_dev, mm_dev = [], []
    for c in range(NCORES):
        a3 = idx_arr[c].reshape(NSTEP, 2 * C, P)
        cols = []
        for q in range(NSTEP):
            cols.append(_wrap_idx(a3[q, 0:cA].reshape(-1)))
            cols.append(_wrap_idx(a3[q, cA : 2 * cA].reshape(-1)))
            cols.append(_wrap_idx(a3[q, 2 * cA : 2 * cA + cB].reshape(-1)))
            cols.append(_wrap_idx(a3[q, 2 * cA + cB :].reshape(-1)))
        idx_dev.append(np.hstack(cols))  # [128, NSTEP*2C*8]
        mm = np.zeros((P, NSTEP * NCH * P), FP8)
        kk = np.arange(NSTEP * NCH * P)
        mm[kk % P, (kk // P) * P + dl_arr[c]] = mm_arr[c].astype(FP8)
        # self (diag) chunks: mm[e, (q*NCH + 2C + sub)*P + e] = diag[node]
        nodes = np.arange(counts[c]) + core_start[c]
        s = slot[nodes]
        t_of = s >> 7
        e_of = s & 127
        q_of = t_of // 2
        sub_of = t_of % 2
        mm[e_of, (q_of * NCH + 2 * C + sub_of) * P + e_of] = diag[nodes].astype(FP8)
        mm_dev.append(mm)

    # x tables (fp8, 256-wide rows, 64 used)
    xg = np.zeros((NCORES * NL, 256), FP8)
    xg[node_core * NL + slot, :FIN] = x.astype(FP8)
    xgA = np.ascontiguousarray(xg[: 4 * NL])
    xgB = np.ascontiguousarray(xg[4 * NL :])
    xloc8 = [
        np.ascontiguousarray(xg[c * NL : (c + 1) * NL]) for c in range(NCORES)
    ]
    xg_nm = np.zeros((NCORES * NL, P), BF16)
    xg_nm[node_core * NL + slot, :FIN] = x.astype(BF16)
    xloc = [np.ascontiguousarray(xg_nm[c * NL : (c + 1) * NL]) for c in range(NCORES)]

    # pooling windows (bf16 bounce_pool, local rows)
    gcnt = np.bincount(batch, minlength=G).astype(np.int64)
    assert gcnt.min() > 0
    GW = int(np.ceil(gcnt.max() / 16) * 16)
    n_calls = GPC_OUT // POOL_GPC
    pidx = []
    cntr = []
    for c in range(NCORES):
        zrow = int(counts[c])
        mean_cols, max_cols = [], []
        for call in range(n_calls):
            mvals = np.zeros(POOL_GPC * GW, np.int64)
            xvals = np.zeros(POOL_GPC * GW, np.int64)
            for gg in range(POOL_GPC):
                g_id = c * GPC_OUT + call * POOL_GPC + gg
                lo = np.searchsorted(batch, g_id, "left")
                hi = np.searchsorted(batch, g_id, "right")
                rows = slot[lo:hi]
                k = hi - lo
                mvals[gg * GW : gg * GW + k] = rows
                mvals[gg * GW + k : (gg + 1) * GW] = zrow
                xvals[gg * GW : gg * GW + k] = rows
                xvals[gg * GW + k : (gg + 1) * GW] = rows[0]
            mean_cols.append(_wrap_idx(mvals))
            max_cols.append(_wrap_idx(xvals))
        pidx.append(np.hstack(mean_cols + max_cols))
        cr = (1.0 / np.maximum(gcnt[c * GPC_OUT : (c + 1) * GPC_OUT], 1.0)).astype(
            np.float32
        )
        cntr.append(np.tile(cr.reshape(1, GPC_OUT), (P, 1)))

    return dict(
        NL=NL, NT=NT, NSTEP=NSTEP, C=C, cA=cA, cB=cB, NCH=NCH, GW=GW,
        idx=idx_dev, mm=mm_dev, xgA=xgA, xgB=xgB, xloc=xloc, xloc8=xloc8,
        pidx=pidx, cntr=cntr,
    )


def _pack_weights(W1, W2, W3, W4, b1, b2, b3, b4, fc_w, fc_b):
    def cheb_pack(W, kin_chunks):
        Wp = np.stack([W[0] - W[2], W[1], 2.0 * W[2]]).astype(np.float32)
        out = np.zeros((3, kin_chunks, P, 256), np.float32)
        fin = W.shape[1]
        for ki in range(kin_chunks):
            lo = ki * P
            hi = min(fin, lo + P)
            if hi > lo:
                out[:, ki, : hi - lo, :H] = Wp[:, lo:hi, :]
        return out.astype(BF16)

    w1 = cheb_pack(W1, 1)
    w2 = cheb_pack(W2, 2)
    w3 = cheb_pack(W3, 2)
    w4 = cheb_pack(W4, 2)
    bvec = np.zeros((P, 4, 2), np.float32)
    for li, b in enumerate([b1, b2, b3, b4]):
        for fo in range(2):
            seg = b[fo * P : min(H, (fo + 1) * P)]
            bvec[: len(seg), li, fo] = seg
    fcw = np.zeros((P, 4, 2), np.float32)
    fcw[:, 0] = fc_w[0:P]
    fcw[: H - P, 1] = fc_w[P:H]
    fcw[:, 2] = fc_w[H : H + P]
    fcw[: H - P, 3] = fc_w[H + P : 2 * H]
    fcb = np.tile(fc_b.astype(np.float32).reshape(1, 2), (GPC_OUT, 1))
    return dict(
        w1=w1, w2=w2, w3=w3, w4=w4, bvec=bvec, fcw=fcw.astype(BF16), fcb=fcb
    )


def _build(NL, NT, NSTEP, C, cA, cB, NCH, GW):
    F32, BF, F8, I16 = (
        mybir.dt.float32, mybir.dt.bfloat16, mybir.dt.float8e4, mybir.dt.int16
    )
    nc = bacc.Bacc(None, num_devices=NCORES, num_swdge_queues=4)
    rg = [list(range(NCORES))]
    n_calls = GPC_OUT // POOL_GPC

    d_xgA = nc.declare_dram_parameter("xgA", [4 * NL, 256], F8, isOutput=False)
    d_xgB = nc.declare_dram_parameter("xgB", [4 * NL, 256], F8, isOutput=False)
    d_xloc8 = nc.declare_dram_parameter("xloc8", [NL, 256], F8, isOutput=False)
    d_xloc = nc.declare_dram_parameter("xloc", [NL, P], BF, isOutput=False)
    d_idx = nc.declare_dram_parameter("idx", [P, NSTEP * 2 * C * 8], I16, isOutput=False)
    d_mm = nc.declare_dram_parameter("mm", [P, NSTEP * NCH * P], F8, isOutput=False)
    d_pidx = nc.declare_dram_parameter(
        "pidx", [P, 2 * n_calls * POOL_GPC * GW // 16], I16, isOutput=False
    )
    d_cntr = nc.declare_dram_parameter("cntr", [P, GPC_OUT], F32, isOutput=False)
    d_w1 = nc.declare_dram_parameter("w1", [3, 1, P, 256], BF, isOutput=False)
    d_w2 = nc.declare_dram_parameter("w2", [3, 2, P, 256], BF, isOutput=False)
    d_w3 = nc.declare_dram_parameter("w3", [3, 2, P, 256], BF, isOutput=False)
    d_w4 = nc.declare_dram_parameter("w4", [3, 2, P, 256], BF, isOutput=False)
    d_bvec = nc.declare_dram_parameter("bvec", [P, 4, 2], F32, isOutput=False)
    d_fcw = nc.declare_dram_parameter("fcw", [P, 4, 2], BF, isOutput=False)
    d_fcb = nc.declare_dram_parameter("fcb", [GPC_OUT, 2], F32, isOutput=False)
    d_out = nc.declare_dram_parameter("out", [GPC_OUT, 2], F32, isOutput=True)

    bounce = nc.dram_tensor("bounce", [NL, 256], F8)  # h (dense out, fp8)
    t1b = nc.dram_tensor("t1b", [NL, 256], F8)  # t1 (prop1 out, fp8)
    bounce_pool = nc.dram_tensor("bounce_pool", [NL, 256], BF)  # last h
    hg = nc.dram_tensor("hg", [NCORES * NL, 256], F8, addr_space="Shared")
    t1g = nc.dram_tensor("t1g", [NCORES * NL, 256], F8, addr_space="Shared")

    with tile.TileContext(nc) as tc:
        with (
            tc.tile_pool(name="const", bufs=1) as cp,
            tc.tile_pool(name="big", bufs=1) as bigp,
            tc.tile_pool(name="work", bufs=3) as wp,
            tc.tile_pool(name="w8", bufs=3) as w8p,
            tc.tile_pool(name="ypool", bufs=2) as yp,
            tc.tile_pool(name="ygath", bufs=4) as ygp,
            tc.tile_pool(name="mpool", bufs=3) as mp,
            tc.tile_pool(name="psA", bufs=3, space="PSUM") as psA,
            tc.tile_pool(name="psB", bufs=2, space="PSUM") as psB,
            tc.tile_pool(name="psC", bufs=2, space="PSUM") as psC,
            tc.tile_pool(name="psD", bufs=1, space="PSUM") as psD,
        ):
            idx_sb = cp.tile([P, NSTEP * 2 * C * 8], I16)
            nc.sync.dma_start(out=idx_sb[:], in_=d_idx.ap())
            pidx_sb = cp.tile([P, 2 * n_calls * POOL_GPC * GW // 16], I16)
            nc.sync.dma_start(out=pidx_sb[:], in_=d_pidx.ap())
            w_sb = {}
            for nm_, dp, kc in (
                ("w1", d_w1, 1), ("w2", d_w2, 2), ("w3", d_w3, 2), ("w4", d_w4, 2)
            ):
                t = cp.tile([P, 3, kc, 256], BF, tag=nm_)
                nc.sync.dma_start(
                    out=t[:], in_=dp.ap().rearrange("t k p f -> p t k f")
                )
                w_sb[nm_] = t
            bvec_sb = cp.tile([P, 4, 2], F32)
            nc.sync.dma_start(out=bvec_sb[:], in_=d_bvec.ap())
            fcw_sb = cp.tile([P, 4, 2], BF)
            nc.sync.dma_start(out=fcw_sb[:], in_=d_fcw.ap())
            fcb_sb = cp.tile([GPC_OUT, 2], F32)
            nc.sync.dma_start(out=fcb_sb[:], in_=d_fcb.ap())
            cntr_sb = cp.tile([P, GPC_OUT], F32)
            nc.sync.dma_start(out=cntr_sb[:], in_=d_cntr.ap())

            ident = cp.tile([P, P], BF)
            make_identity(nc, ident[:])

            hT = [
                bigp.tile([P, 2, NT * P], BF, tag=f"hT{i}", name=f"hT{i}")
                for i in range(2)
            ]
            t1T = bigp.tile([P, 2, NT * P], BF, tag="t1T")
            p2T = bigp.tile([P, 2, NT * P], BF, tag="p2T")
            for buf in (hT[0], hT[1], t1T, p2T):
                nc.vector.memset(buf[:], 0.0)
            zt = cp.tile([P, 256], BF)
            nc.vector.memset(zt[:], 0.0)
            for t in range(NT):
                nc.sync.dma_start(
                    out=bounce_pool.ap()[t * P : (t + 1) * P, :], in_=zt[:]
                )

            for t in range(NT):
                xt = wp.tile([P, P], BF, tag="xload")
                nc.sync.dma_start(out=xt[:], in_=d_xloc.ap()[t * P : (t + 1) * P, :])
                pt = psB.tile([P, P], BF, tag="tp")
                nc.tensor.transpose(pt[:], xt[:], ident[:])
                nc.vector.tensor_copy(out=hT[0][:, 0, t * P : (t + 1) * P], in_=pt[:])

            def allgather(src, dst):
                nc.gpsimd.collective_compute(
                    "AllGather",
                    mybir.AluOpType.bypass,
                    replica_groups=rg,
                    ins=[src.ap().opt()],
                    outs=[dst.ap().opt()],
                )

            def prop(tabA_ap, tabB_ap, self_dram, NW, outT, write_t1, tile_cb=None):
                # gather calls: (chunk range, idx col offset units of 8/chunk)
                segs = [
                    (0, cA), (cA, 2 * cA), (2 * cA, 2 * cA + cB), (2 * cA + cB, 2 * C)
                ]
                for q in range(NSTEP):
                    y = ygp.tile([P, NCH, 256], F8, tag="Y")
                    for j, (lo, hi) in enumerate(segs):
                        nc.gpsimd.dma_gather(
                            out_ap=y[:, lo:hi, :],
                            in_ap=tabA_ap if j < 2 else tabB_ap,
                            idxs_ap=idx_sb[
                                :, q * 2 * C * 8 + lo * 8 : q * 2 * C * 8 + hi * 8
                            ],
                            num_idxs=(hi - lo) * P,
                            num_idxs_reg=(hi - lo) * P,
                            elem_size=256,
                            single_packet=False,
                            queue_num=(q + j) % 4,
                        )
                    for sub in range(2):
                        t = 2 * q + sub
                        nc.sync.dma_start(
                            out=y[:, 2 * C + sub, :],
                            in_=self_dram.ap()[t * P : (t + 1) * P, :],
                        )
                    mt = mp.tile([P, NCH, P], F8, tag="mt")
                    nc.sync.dma_start(
                        out=mt[:],
                        in_=d_mm.ap()[:, q * NCH * P : (q + 1) * NCH * P].rearrange(
                            "e (k d) -> e k d", d=P
                        ),
                    )
                    for sub in range(2):
                        t = 2 * q + sub
                        chunks = (
                            list(range(sub * cA, (sub + 1) * cA))
                            + list(range(2 * cA + sub * cB, 2 * cA + (sub + 1) * cB))
                            + [2 * C + sub]
                        )
                        acc = psA.tile([P, NW], F32, tag="acc")
                        for ki, ch in enumerate(chunks):
                            nc.tensor.matmul(
                                acc[:],
                                lhsT=mt[:, ch, :],
                                rhs=y[:, ch, 0:NW],
                                start=(ki == 0),
                                stop=(ki == len(chunks) - 1),
                            )
                        nm = wp.tile([P, NW], BF, tag="nm")
                        nc.vector.tensor_copy(out=nm[:], in_=acc[:])
                        if write_t1:
                            nm8 = w8p.tile([P, NW], F8, tag="nm8")
                            nc.scalar.activation(
                                nm8[:], acc[:], mybir.ActivationFunctionType.Copy
                            )
                            nc.sync.dma_start(
                                out=t1b.ap()[t * P : (t + 1) * P, 0:NW], in_=nm8[:]
                            )
                        nfc = (NW + P - 1) // P
                        for fc in range(nfc):
                            w_fc = min(P, NW - fc * P)
                            pt = psB.tile([P, P], BF, tag="tp")
                            nc.tensor.transpose(
                                pt[:w_fc, :],
                                nm[:, fc * P : fc * P + w_fc],
                                ident[:],
                            )
                            nc.vector.tensor_copy(
                                out=outT[:w_fc, fc, t * P : (t + 1) * P],
                                in_=pt[:w_fc, :],
                            )
                        if tile_cb is not None:
                            tile_cb(t)

            def dense_tile(l_idx, wt, kc, inT0, h_out, last, t):
                    terms = [(inT0, 0), (t1T, 1), (p2T, 2)]
                    if last:
                        nmt = wp.tile([P, 256], BF, tag="nmp")
                    else:
                        nmt = w8p.tile([P, 256], F8, tag="nm8d")
                    for fo in range(2):
                        pd = psC.tile([P, P], F32, tag="pd")
                        n_mm = len(terms) * kc
                        i_mm = 0
                        for inT, term in terms:
                            for ki in range(kc):
                                nc.tensor.matmul(
                                    pd[:],
                                    lhsT=w_sb[wt][:, term, ki, fo * P : (fo + 1) * P],
                                    rhs=inT[:, ki, t * P : (t + 1) * P],
                                    start=(i_mm == 0),
                                    stop=(i_mm == n_mm - 1),
                                )
                                i_mm += 1
                        nc.scalar.activation(
                            h_out[:, fo, t * P : (t + 1) * P],
                            pd[:],
                            mybir.ActivationFunctionType.Relu,
                            bias=bvec_sb[:, l_idx, fo : fo + 1],
                        )
                        pt = psB.tile([P, P], BF, tag="tp")
                        nc.tensor.transpose(
                            pt[:], h_out[:, fo, t * P : (t + 1) * P], ident[:]
                        )
                        if last:
                            nc.vector.tensor_copy(
                                out=nmt[:, fo * P : (fo + 1) * P], in_=pt[:]
                            )
                        else:
                            nc.scalar.activation(
                                nmt[:, fo * P : (fo + 1) * P], pt[:],
                                mybir.ActivationFunctionType.Copy,
                            )
                    dst = bounce_pool if last else bounce
                    nc.sync.dma_start(
                        out=dst.ap()[t * P : (t + 1) * P, :], in_=nmt[:]
                    )

            # ================= layer 1 (input x, width 64->200) =================
            prop(d_xgA.ap(), d_xgB.ap(), d_xloc8, FIN, t1T, True)
            allgather(t1b, t1g)
            prop(
                t1g.ap()[0 : 4 * NL, :], t1g.ap()[4 * NL :, :], t1b, FIN, p2T,
                False,
                tile_cb=lambda t: dense_tile(0, "w1", 1, hT[0], hT[1], False, t),
            )
            allgather(bounce, hg)

            # ================= layers 2..4 =================
            cur = 1
            for li, wt in ((1, "w2"), (2, "w3"), (3, "w4")):
                prop(hg.ap()[0 : 4 * NL, :], hg.ap()[4 * NL :, :], bounce, H, t1T, True)
                allgather(t1b, t1g)
                prop(
                    t1g.ap()[0 : 4 * NL, :], t1g.ap()[4 * NL :, :], t1b, H, p2T,
                    False,
                    tile_cb=(
                        lambda t, li=li, wt=wt, cur=cur: dense_tile(
                            li, wt, 2, hT[cur], hT[1 - cur], li == 3, t
                        )
                    ),
                )
                cur = 1 - cur
                if li < 3:
                    allgather(bounce, hg)

            # ================= pooling + fc + log_softmax =================
            red = {}
            for typ in range(2):
                rT = bigp.tile([P, 2, GPC_OUT], F32, tag=f"red{typ}")
                red[typ] = rT
                for call in range(n_calls):
                    py = yp.tile([P, 2, POOL_GPC * GW], BF, tag="poolY")
                    base = (typ * n_calls + call) * (POOL_GPC * GW // 16)
                    nc.gpsimd.dma_gather(
                        out_ap=py[:],
                        in_ap=bounce_pool.ap(),
                        idxs_ap=pidx_sb[:, base : base + POOL_GPC * GW // 16],
                        num_idxs=POOL_GPC * GW,
                        num_idxs_reg=POOL_GPC * GW,
                        elem_size=256,
                        transpose=True,
                        single_packet=False,
                    )
                    for gg in range(POOL_GPC):
                        gcol = call * POOL_GPC + gg
                        for fc in range(2):
                            nc.vector.tensor_reduce(
                                out=rT[:, fc, gcol : gcol + 1],
                                in_=py[:, fc, gg * GW : (gg + 1) * GW],
                                axis=mybir.AxisListType.X,
                                op=mybir.AluOpType.add
                                if typ == 0
                                else mybir.AluOpType.max,
                            )
            pool_bf = bigp.tile([P, 4, GPC_OUT], BF, tag="poolbf")
            for fc in range(2):
                nc.vector.tensor_tensor(
                    out=pool_bf[:, fc, :],
                    in0=red[0][:, fc, :],
                    in1=cntr_sb[:, :],
                    op=mybir.AluOpType.mult,
                )
                nc.vector.tensor_copy(out=pool_bf[:, 2 + fc, :], in_=red[1][:, fc, :])
            pfc = psD.tile([GPC_OUT, 2], F32, tag="pfc")
            for j in range(4):
                nc.tensor.matmul(
                    pfc[:],
                    lhsT=pool_bf[:, j, :],
                    rhs=fcw_sb[:, j, :],
                    start=(j == 0),
                    stop=(j == 3),
                )
            z = wp.tile([GPC_OUT, 2], F32, tag="z")
            nc.vector.tensor_tensor(
                out=z[:], in0=pfc[:], in1=fcb_sb[:, :], op=mybir.AluOpType.add
            )
            mx = wp.tile([GPC_OUT, 1], F32, tag="mx")
            nc.vector.tensor_reduce(
                out=mx[:], in_=z[:], axis=mybir.AxisListType.X, op=mybir.AluOpType.max
            )
            zm = wp.tile([GPC_OUT, 2], F32, tag="zm")
            nc.vector.tensor_scalar(
                zm[:], z[:], mx[:], None, mybir.AluOpType.subtract
            )
            ez = wp.tile([GPC_OUT, 2], F32, tag="ez")
            nc.scalar.activation(ez[:], zm[:], mybir.ActivationFunctionType.Exp)
            sz = wp.tile([GPC_OUT, 1], F32, tag="sz")
            nc.vector.tensor_reduce(
                out=sz[:], in_=ez[:], axis=mybir.AxisListType.X, op=mybir.AluOpType.add
            )
            lz = wp.tile([GPC_OUT, 1], F32, tag="lz")
            nc.scalar.activation(lz[:], sz[:], mybir.ActivationFunctionType.Ln)
            oz = wp.tile([GPC_OUT, 2], F32, tag="oz")
            nc.vector.tensor_scalar(
                oz[:], zm[:], lz[:], None, mybir.AluOpType.subtract
            )
            nc.sync.dma_start(out=d_out.ap(), in_=oz[:])

    nc.finalize()
    return nc


def kernel(**inputs):
    x = np.asarray(inputs["x"], np.float32)
    edge_index = np.asarray(inputs["edge_index"])
    batch = np.asarray(inputs["batch"])
    lmax = np.asarray(inputs["lmax"], np.float32)

    pp = _preprocess(x, edge_index, batch, lmax)
    wts = _pack_weights(
        np.asarray(inputs["W1"], np.float32), np.asarray(inputs["W2"], np.float32),
        np.asarray(inputs["W3"], np.float32), np.asarray(inputs["W4"], np.float32),
        np.asarray(inputs["b1"], np.float32), np.asarray(inputs["b2"], np.float32),
        np.asarray(inputs["b3"], np.float32), np.asarray(inputs["b4"], np.float32),
        np.asarray(inputs["fc_w"], np.float32), np.asarray(inputs["fc_b"], np.float32),
    )

    key = (
        pp["NL"], pp["NT"], pp["NSTEP"], pp["C"], pp["cA"], pp["cB"],
        pp["NCH"], pp["GW"],
    )
    if key not in _cache:
        _cache[key] = _build(*key)
    nc = _cache[key]

    shared = dict(
        xgA=pp["xgA"], xgB=pp["xgB"],
        w1=wts["w1"], w2=wts["w2"], w3=wts["w3"], w4=wts["w4"],
        bvec=wts["bvec"], fcw=wts["fcw"], fcb=wts["fcb"],
    )
    in_maps = [
        dict(
            shared,
            xloc=pp["xloc"][c], xloc8=pp["xloc8"][c],
            idx=pp["idx"][c], mm=pp["mm"][c],
            pidx=pp["pidx"][c], cntr=pp["cntr"][c],
        )
        for c in range(NCORES)
    ]
    trace = bool(int(__import__("os").environ.get("KERNEL_TRACE", "0")))
    res = run_bass_kernel_spmd(nc, in_maps, list(range(NCORES)), trace=trace)
    if trace:
        kernel.last_exec_time_ns = res.exec_time_ns
        kernel.last_results = res
    out = np.concatenate([res.results[c]["out"] for c in range(NCORES)], axis=0)
    return out.astype(np.float32)


kernel.last_exec_time_ns = None
